# revision 9
# baseline (speedup 1.0000x reference)
"""GPT forward kernel for 8 TRN2 NeuronCores (v2).

Data-parallel over batch (B=8 -> 1 sequence per core). Host folds the LN
affine params into the adjacent weights (exact), pre-transposes weights to
put the contraction dim on SBUF partitions, casts them to bf16, and does
the embedding gather. On device the residual stream is kept transposed
(x^T [D, T] fp32 in SBUF); LayerNorm is pure normalization whose stats
matmuls are interleaved into the producing GEMM, broadcast planes are
built on GpSimd, and PSUM drains ride the Scalar engine so the Tensor
engine stays dense.
"""
import sys
sys.path.insert(0, '/opt/trn_rl_repo')
import numpy as np
import ml_dtypes

import concourse.bass as bass
import concourse.tile as tile
from concourse import bacc, mybir
from concourse.bass_utils import run_bass_kernel_spmd

B, T, D, H, L, V, MAXT = 8, 1024, 1024, 16, 8, 8192, 4096
HD = D // H          # 64
P = 128
DS = D // P          # 8 d-subtiles
TS = T // P          # 8 t-subtiles
D2S = (2 * D) // P   # 16 mlp subtiles
VS = V // 512        # 16 vocab chunks
NCH = 512
EPS = 1e-5
SCALE = 1.0 / np.sqrt(HD)

F32 = mybir.dt.float32
BF16 = mybir.dt.bfloat16
AF = mybir.ActivationFunctionType
ALU = mybir.AluOpType

# smalls[:, col] layout, per layer base = l*32  (bo, b2: [P,1]-packed cols)
SM_BO, SM_B2, SM_B1 = 0, 8, 16
SM_PER_LAYER = 32
SM_COLS = L * SM_PER_LAYER

TRACE = False
LAST_RESULTS = None


def _build(repeat=1):
    import contextlib
    nc = bacc.Bacc("TRN2", target_bir_lowering=False)

    x0T_d = nc.dram_tensor("x0T", [D, T], F32, kind="ExternalInput")
    WqT_d = nc.dram_tensor("WqT", [L, D, D], BF16, kind="ExternalInput")
    WkT_d = nc.dram_tensor("WkT", [L, D, D], BF16, kind="ExternalInput")
    WvT_d = nc.dram_tensor("WvT", [L, D, D], BF16, kind="ExternalInput")
    WoT_d = nc.dram_tensor("WoT", [L, D, D], BF16, kind="ExternalInput")
    W1T_d = nc.dram_tensor("W1T", [L, D, 2 * D], BF16, kind="ExternalInput")
    W2T_d = nc.dram_tensor("W2T", [L, 2 * D, D], BF16, kind="ExternalInput")
    hT_w_d = nc.dram_tensor("headT", [D, V], BF16, kind="ExternalInput")
    sm_d = nc.dram_tensor("smalls", [P, SM_COLS], F32, kind="ExternalInput")
    brow_d = nc.dram_tensor("brows", [L, 2 * D], BF16, kind="ExternalInput")
    bv_d = nc.dram_tensor("bvB", [1, L * D], BF16, kind="ExternalInput")
    hb_d = nc.dram_tensor("hbB", [1, V], BF16, kind="ExternalInput")
    mask_d = nc.dram_tensor("mask01", [P, P], BF16, kind="ExternalInput")
    out_d = nc.dram_tensor("logits", [T, V], F32, kind="ExternalOutput")

    out_r = out_d[:, :].rearrange("(t pi) v -> pi t v", pi=P)

    with tile.TileContext(nc) as tc:
        with (
            tc.tile_pool(name="pc", bufs=1) as pc,
            tc.tile_pool(name="pw", bufs=3) as pw,
            tc.tile_pool(name="pbv", bufs=2) as pbv,
            tc.tile_pool(name="pbr", bufs=2) as pbr,
            tc.tile_pool(name="phb", bufs=2) as phb,
            tc.tile_pool(name="ppt", bufs=2) as ppt,
            tc.tile_pool(name="px", bufs=3) as px,
            tc.tile_pool(name="pr", bufs=3) as pr,
            tc.tile_pool(name="pln", bufs=4) as pln,
            tc.tile_pool(name="pbc", bufs=2) as pbc,
            tc.tile_pool(name="pps", bufs=4, space="PSUM") as pps,
            tc.tile_pool(name="ppo", bufs=2, space="PSUM") as ppo,
            tc.tile_pool(name="pst", bufs=2, space="PSUM") as pst,
        ):
            xT = pc.tile([P, DS, T], F32)     # residual, transposed
            xb = pc.tile([P, DS, T], BF16)    # bf16 copy of residual (LN input)
            hT = pc.tile([P, DS, T], BF16)    # LN output; reused as attn y^T
            qkT = pc.tile([P, 2 * DS, T], BF16)  # q rows 0:8, k rows 8:16; reused as gT
            Vg = pc.tile([P, TS, H, HD + 1], BF16)
            yT = hT
            sm = pc.tile([P, SM_COLS], F32)
            mask = pc.tile([P, P], BF16)
            ones_row = pc.tile([1, NCH], BF16)
            ones_cb = pc.tile([P, 1], BF16)
            eps_t = pc.tile([1, 1], F32)

            nc.vector.memset(ones_row[:], 1.0)
            nc.vector.memset(ones_cb[:], 1.0)
            nc.vector.memset(eps_t[:], EPS)
            nc.vector.memset(Vg[:, :, :, HD:HD + 1], 1.0)
            nc.sync.dma_start(sm[:], sm_d[:, :])
            nc.sync.dma_start(mask[:], mask_d[:, :])

            def ln_stats_k(stat2c, c, k):
                """stat2c: PSUM [65, NCH]; row 0 accumulates sum, row 64 sumsq."""
                tch = bass.ts(c, NCH)
                nc.vector.tensor_copy(xb[:, k, tch], xT[:, k, tch])
                sq = px.tile([P, NCH], BF16, tag="sq")
                nc.scalar.activation(sq[:], xb[:, k, tch], AF.Square)
                nc.tensor.matmul(stat2c[0:1, :], ones_cb[:],
                                 xb[:, k, tch], start=(k == 0),
                                 stop=(k == DS - 1), skip_group_check=True)
                nc.tensor.matmul(stat2c[64:65, :], ones_cb[:],
                                 sq[:], start=(k == 0), stop=(k == DS - 1),
                                 skip_group_check=True)

            def ln_finish(stat2c):
                """-> (A, B) bf16 [P, NCH] planes: h = x*A + B."""
                m2 = pr.tile([1, NCH], F32, tag="r")
                nc.scalar.activation(m2[:], stat2c[0:1, :],
                                     AF.Square, scale=1.0 / D)
                var = pr.tile([1, NCH], F32, tag="r")
                nc.vector.scalar_tensor_tensor(var[:], stat2c[64:65, :],
                                               1.0 / D, m2[:],
                                               op0=ALU.mult, op1=ALU.subtract)
                sd = pr.tile([1, NCH], F32, tag="r")
                nc.scalar.activation(sd[:], var[:], AF.Sqrt, bias=eps_t[:])
                srow = pr.tile([1, NCH], BF16, tag="rb")
                nm = pr.tile([1, NCH], BF16, tag="rb")
                with nc.allow_low_precision(reason="LN planes applied in bf16 anyway"):
                    nc.vector.reciprocal(srow[:], sd[:])
                    nc.vector.scalar_tensor_tensor(nm[:], stat2c[0:1, :],
                                                   -1.0 / D, srow[:],
                                                   op0=ALU.mult, op1=ALU.mult)
                A = pln.tile([P, NCH], BF16, tag="pl")
                nc.gpsimd.partition_broadcast(A[:], srow[:], channels=P)
                Bp = pln.tile([P, NCH], BF16, tag="pl")
                nc.gpsimd.partition_broadcast(Bp[:], nm[:], channels=P)
                return A, Bp

            def ln_apply(A, Bp, c):
                tch = bass.ts(c, NCH)
                for k in range(DS):
                    tmp = px.tile([P, NCH], BF16, tag="lt")
                    nc.vector.tensor_mul(tmp[:], xb[:, k, tch], A[:])
                    nc.vector.tensor_add(hT[:, k, tch], tmp[:], Bp[:])

            loop_cm = tc.For_i(0, repeat, 1) if repeat > 1 else contextlib.nullcontext()
            with loop_cm:
                nc.sync.dma_start(xT[:], x0T_d[:, :].rearrange("(po pi) t -> pi po t", pi=P))
                stat0 = pst.tile([65, NCH], F32, tag="st")
                stat1 = pst.tile([65, NCH], F32, tag="st")
                stat = [stat0, stat1]
                for c in range(2):
                    for k in range(DS):
                        ln_stats_k(stat[c], c, k)

                for l in range(L):
                    base = l * SM_PER_LAYER
                    br_t = pbr.tile([1, 2 * D], BF16, tag="br")
                    nc.sync.dma_start(br_t[:], brow_d[l:l + 1, :])

                    # ---- LN1 finish + apply ----
                    for c in range(2):
                        A, Bp = ln_finish(stat[c])
                        ln_apply(A, Bp, c)

                    # ---- q^T / k^T projections (bias via K=1 matmul row) ----
                    for which, W_d in ((0, WqT_d), (1, WkT_d)):
                        qoff = which * DS
                        for half in range(2):
                            wsl = pw.tile([P, DS, NCH], BF16, tag="w")
                            nc.sync.dma_start(
                                wsl[:],
                                W_d[l].rearrange("(po pi) o -> pi po o", pi=P)[:, :, bass.ts(half, NCH)])
                            for m in range(4):
                                mo = half * 4 + m
                                bsl = br_t[0:1, which * D + mo * P: which * D + (mo + 1) * P]
                                ps0 = pps.tile([P, NCH], F32, tag="a")
                                ps1 = pps.tile([P, NCH], F32, tag="a")
                                nc.tensor.matmul(ps0[:], bsl, ones_row[:], start=True, stop=False)
                                nc.tensor.matmul(ps1[:], bsl, ones_row[:], start=True, stop=False)
                                for k in range(DS):
                                    nc.tensor.matmul(ps0[:], wsl[:, k, bass.ts(m, P)],
                                                     hT[:, k, bass.ts(0, NCH)],
                                                     start=False, stop=(k == DS - 1))
                                    nc.tensor.matmul(ps1[:], wsl[:, k, bass.ts(m, P)],
                                                     hT[:, k, bass.ts(1, NCH)],
                                                     start=False, stop=(k == DS - 1))
                                nc.scalar.activation(qkT[:, qoff + mo, bass.ts(0, NCH)], ps0[:], AF.Copy)
                                nc.scalar.activation(qkT[:, qoff + mo, bass.ts(1, NCH)], ps1[:], AF.Copy)

                    # ---- V projection: out[t, o] into Vg ----
                    for half in range(2):
                        wsl = pw.tile([P, DS, NCH], BF16, tag="w")
                        nc.sync.dma_start(
                            wsl[:],
                            WvT_d[l].rearrange("(po pi) o -> pi po o", pi=P)[:, :, bass.ts(half, NCH)])
                        bvs = pbv.tile([1, NCH], BF16, tag="bvs")
                        nc.sync.dma_start(bvs[:], bv_d[:, l * D + half * NCH:l * D + (half + 1) * NCH])
                        for t_ in range(TS):
                            ps_t = pps.tile([P, NCH], F32, tag="a")
                            nc.tensor.matmul(ps_t[:], ones_row[0:1, 0:P], bvs[:],
                                             start=True, stop=False)
                            for k in range(DS):
                                nc.tensor.matmul(ps_t[:], hT[:, k, bass.ts(t_, P)],
                                                 wsl[:, k, :],
                                                 start=False, stop=(k == DS - 1))
                            nc.scalar.activation(
                                Vg[:, t_, 8 * half:8 * half + 8, 0:HD],
                                ps_t[:].rearrange("p (h d) -> p h d", d=HD), AF.Copy)

                    # ---- attention, per (head, q-chunk) ----
                    for h in range(H):
                        pbase = (h % 2) * HD
                        sub = h // 2
                        for c in range(2):
                            tch = bass.ts(c, NCH)
                            PT = ppt.tile([P, TS, NCH], BF16, tag="pt")
                            ntk = 4 * c + 4
                            for tk in range(ntk):
                                ls = max(0, tk * P - c * NCH)
                                w_ = NCH - ls
                                sT = pps.tile([P, NCH], F32, tag="a")
                                nc.tensor.matmul(
                                    sT[:, :w_],
                                    qkT[pbase:pbase + HD, DS + sub, bass.ts(tk, P)],
                                    qkT[pbase:pbase + HD, sub, c * NCH + ls:(c + 1) * NCH],
                                    start=True, stop=True)
                                nc.scalar.activation(PT[:, tk, ls:], sT[:, :w_], AF.Exp,
                                                     scale=float(SCALE))
                                if tk >= 4 * c:
                                    nc.vector.tensor_mul(PT[:, tk, ls:ls + P],
                                                         PT[:, tk, ls:ls + P], mask[:])
                            po = ppo.tile([HD + 1, NCH], F32, tag="o")
                            for tk in range(ntk):
                                ls = max(0, tk * P - c * NCH)
                                nc.tensor.matmul(po[:, ls:], Vg[:, tk, h, :],
                                                 PT[:, tk, ls:],
                                                 start=(tk == 0), stop=(tk == ntk - 1))
                            dn = pr.tile([1, NCH], F32, tag="dn")
                            nc.vector.reciprocal(dn[:], po[HD:HD + 1, :])
                            bc = pbc.tile([HD, NCH], F32, tag="bc")
                            nc.gpsimd.partition_broadcast(bc[:], dn[:], channels=HD)
                            nc.vector.tensor_mul(yT[pbase:pbase + HD, sub, tch],
                                                 po[0:HD, :], bc[:])

                    # ---- attention out projection + residual + LN2 stats ----
                    wo0 = pw.tile([P, DS, NCH], BF16, tag="w")
                    nc.sync.dma_start(wo0[:], WoT_d[l].rearrange("(po pi) o -> pi po o", pi=P)[:, :, bass.ts(0, NCH)])
                    wo1 = pw.tile([P, DS, NCH], BF16, tag="w")
                    nc.sync.dma_start(wo1[:], WoT_d[l].rearrange("(po pi) o -> pi po o", pi=P)[:, :, bass.ts(1, NCH)])
                    stat2a = pst.tile([65, NCH], F32, tag="st")
                    stat2b = pst.tile([65, NCH], F32, tag="st")
                    stat2 = [stat2a, stat2b]
                    for mo in range(DS):
                        wsl_o = wo0 if mo < 4 else wo1
                        m = mo % 4
                        ps0 = pps.tile([P, NCH], F32, tag="a")
                        ps1 = pps.tile([P, NCH], F32, tag="a")
                        for k in range(DS):
                            nc.tensor.matmul(ps0[:], wsl_o[:, k, bass.ts(m, P)],
                                             yT[:, k, bass.ts(0, NCH)],
                                             start=(k == 0), stop=(k == DS - 1))
                            nc.tensor.matmul(ps1[:], wsl_o[:, k, bass.ts(m, P)],
                                             yT[:, k, bass.ts(1, NCH)],
                                             start=(k == 0), stop=(k == DS - 1))
                        for c, psx in ((0, ps0), (1, ps1)):
                            tch = bass.ts(c, NCH)
                            nc.vector.scalar_tensor_tensor(
                                xT[:, mo, tch], psx[:],
                                sm[:, base + SM_BO + mo:base + SM_BO + mo + 1],
                                xT[:, mo, tch], op0=ALU.add, op1=ALU.add)
                            ln_stats_k(stat2[c], c, mo)

                    # ---- LN2 finish + apply ----
                    for c in range(2):
                        A, Bp = ln_finish(stat2[c])
                        ln_apply(A, Bp, c)

                    # ---- MLP W1 -> gelu -> gT (qkT reused) ----
                    for quarter in range(4):
                        wsl = pw.tile([P, DS, NCH], BF16, tag="w")
                        nc.sync.dma_start(
                            wsl[:],
                            W1T_d[l].rearrange("(po pi) o -> pi po o", pi=P)[:, :, bass.ts(quarter, NCH)])
                        for m in range(4):
                            mo = quarter * 4 + m
                            ps0 = pps.tile([P, NCH], F32, tag="a")
                            ps1 = pps.tile([P, NCH], F32, tag="a")
                            for k in range(DS):
                                nc.tensor.matmul(ps0[:], wsl[:, k, bass.ts(m, P)],
                                                 hT[:, k, bass.ts(0, NCH)],
                                                 start=(k == 0), stop=(k == DS - 1))
                                nc.tensor.matmul(ps1[:], wsl[:, k, bass.ts(m, P)],
                                                 hT[:, k, bass.ts(1, NCH)],
                                                 start=(k == 0), stop=(k == DS - 1))
                            for c, psx in ((0, ps0), (1, ps1)):
                                nc.scalar.activation(
                                    qkT[:, mo, bass.ts(c, NCH)], psx[:], AF.Gelu,
                                    bias=sm[:, base + SM_B1 + mo:base + SM_B1 + mo + 1])

                    # ---- W2 + residual + next-LN stats ----
                    stat3a = pst.tile([65, NCH], F32, tag="st")
                    stat3b = pst.tile([65, NCH], F32, tag="st")
                    stat3 = [stat3a, stat3b]
                    for quarter in range(4):
                        w2q = pw.tile([P, D2S, P * 2], BF16, tag="w")
                        nc.sync.dma_start(
                            w2q[:],
                            W2T_d[l].rearrange("(po pi) o -> pi po o", pi=P)[:, :, bass.ts(quarter, P * 2)])
                        for m in range(2):
                            mo = quarter * 2 + m
                            ps0 = pps.tile([P, NCH], F32, tag="a")
                            ps1 = pps.tile([P, NCH], F32, tag="a")
                            for k in range(D2S):
                                nc.tensor.matmul(ps0[:], w2q[:, k, bass.ts(m, P)],
                                                 qkT[:, k, bass.ts(0, NCH)],
                                                 start=(k == 0), stop=(k == D2S - 1))
                                nc.tensor.matmul(ps1[:], w2q[:, k, bass.ts(m, P)],
                                                 qkT[:, k, bass.ts(1, NCH)],
                                                 start=(k == 0), stop=(k == D2S - 1))
                            for c, psx in ((0, ps0), (1, ps1)):
                                tch = bass.ts(c, NCH)
                                nc.vector.scalar_tensor_tensor(
                                    xT[:, mo, tch], psx[:],
                                    sm[:, base + SM_B2 + mo:base + SM_B2 + mo + 1],
                                    xT[:, mo, tch], op0=ALU.add, op1=ALU.add)
                                ln_stats_k(stat3[c], c, mo)
                    stat = stat3

                # ---- final LN + vocab head ----
                for c in range(2):
                    A, Bp = ln_finish(stat[c])
                    ln_apply(A, Bp, c)
                hw_r = hT_w_d[:, :].rearrange("(po pi) v -> pi po v", pi=P)
                for vp in range(VS // 2):
                    ws0 = pw.tile([P, DS, NCH], BF16, tag="w")
                    nc.sync.dma_start(ws0[:], hw_r[:, :, bass.ts(2 * vp, NCH)])
                    ws1 = pw.tile([P, DS, NCH], BF16, tag="w")
                    nc.sync.dma_start(ws1[:], hw_r[:, :, bass.ts(2 * vp + 1, NCH)])
                    hb2 = phb.tile([1, 2 * NCH], BF16, tag="hb")
                    nc.sync.dma_start(hb2[:], hb_d[:, 2 * vp * NCH:(2 * vp + 2) * NCH])
                    for t_ in range(TS):
                        ps0 = pps.tile([P, NCH], F32, tag="a")
                        ps1 = pps.tile([P, NCH], F32, tag="a")
                        nc.tensor.matmul(ps0[:], ones_row[0:1, 0:P], hb2[0:1, 0:NCH],
                                         start=True, stop=False)
                        nc.tensor.matmul(ps1[:], ones_row[0:1, 0:P], hb2[0:1, NCH:2 * NCH],
                                         start=True, stop=False)
                        for k in range(DS):
                            nc.tensor.matmul(ps0[:], hT[:, k, bass.ts(t_, P)],
                                             ws0[:, k, :],
                                             start=False, stop=(k == DS - 1))
                            nc.tensor.matmul(ps1[:], hT[:, k, bass.ts(t_, P)],
                                             ws1[:, k, :],
                                             start=False, stop=(k == DS - 1))
                        for j, psx in ((0, ps0), (1, ps1)):
                            ot = px.tile([P, NCH], F32, tag="ot")
                            nc.scalar.activation(ot[:], psx[:], AF.Copy)
                            nc.sync.dma_start(out_r[:, t_, bass.ts(2 * vp + j, NCH)], ot[:])

    nc.compile()
    return nc


_NC = {}


def _get_nc(repeat=1):
    if repeat not in _NC:
        _NC[repeat] = _build(repeat)
    return _NC[repeat]


def _pack_cols(vec, ncols):
    """[ncols*128] -> [128, ncols] with column j = vec[j*128:(j+1)*128]."""
    return np.ascontiguousarray(vec.reshape(ncols, P).T)


def kernel(idx, timesteps, tok_emb_w, pos_emb, global_pos_emb,
           ln1_w, ln1_b, Wq, bq, Wk, bk, Wv, bv, Wo, bo,
           ln2_w, ln2_b, W1, b1, W2, b2, lnf_w, lnf_b, head_w):
    global LAST_RESULTS
    f = lambda a: np.asarray(a, dtype=np.float32)
    idx = np.asarray(idx, dtype=np.int64)
    tsteps = np.asarray(timesteps, dtype=np.int64)
    tok_emb_w, pos_emb, global_pos_emb = f(tok_emb_w), f(pos_emb), f(global_pos_emb)

    # embedding on host (tiny compute, avoids on-device gather)
    x0 = tok_emb_w[idx] + global_pos_emb[0][tsteps[:, 0]][:, None, :] + pos_emb[:, :T]
    x0 = np.ascontiguousarray(x0.astype(np.float32))

    # fold LN affine params into adjacent weights (exact rewrite)
    Wq, bq, Wk, bk, Wv, bv = f(Wq), f(bq), f(Wk), f(bk), f(Wv), f(bv)
    Wo, bo, W1, b1, W2, b2 = f(Wo), f(bo), f(W1), f(b1), f(W2), f(b2)
    ln1_w, ln1_b, ln2_w, ln2_b = f(ln1_w), f(ln1_b), f(ln2_w), f(ln2_b)
    lnf_w, lnf_b, head_w = f(lnf_w), f(lnf_b), f(head_w)

    Wqf = Wq * ln1_w[:, None, :]
    bqf = bq + np.einsum('lod,ld->lo', Wq, ln1_b)
    Wkf = Wk * ln1_w[:, None, :]
    bkf = bk + np.einsum('lod,ld->lo', Wk, ln1_b)
    Wvf = Wv * ln1_w[:, None, :]
    bvf = bv + np.einsum('lod,ld->lo', Wv, ln1_b)
    W1f = W1 * ln2_w[:, None, :]
    b1f = b1 + np.einsum('lod,ld->lo', W1, ln2_b)
    headf = head_w * lnf_w[None, :]
    hb = head_w @ lnf_b

    bf = lambda a: np.ascontiguousarray(np.asarray(a, np.float32)).astype(ml_dtypes.bfloat16)
    shared = {
        "WqT": bf(Wqf.transpose(0, 2, 1)),
        "WkT": bf(Wkf.transpose(0, 2, 1)),
        "WvT": bf(Wvf.transpose(0, 2, 1)),
        "WoT": bf(Wo.transpose(0, 2, 1)),
        "W1T": bf(W1f.transpose(0, 2, 1)),
        "W2T": bf(W2.transpose(0, 2, 1)),
        "headT": bf(headf.T),
        "bvB": bf(bvf.reshape(1, L * D)),
        "hbB": bf(hb.reshape(1, V)),
        "brows": bf(np.concatenate([bqf, bkf], axis=1)),  # [L, 2D]
    }
    smalls = np.zeros((P, SM_COLS), np.float32)
    for l in range(L):
        b_ = l * SM_PER_LAYER
        smalls[:, b_ + SM_BO:b_ + SM_BO + 8] = _pack_cols(bo[l], DS)
        smalls[:, b_ + SM_B2:b_ + SM_B2 + 8] = _pack_cols(b2[l], DS)
        smalls[:, b_ + SM_B1:b_ + SM_B1 + 16] = _pack_cols(b1f[l], D2S)
    shared["smalls"] = smalls

    m01 = (np.arange(P)[:, None] <= np.arange(P)[None, :])
    shared["mask01"] = m01.astype(ml_dtypes.bfloat16)

    in_maps = []
    for b_ in range(B):
        m = dict(shared)
        m["x0T"] = np.ascontiguousarray(x0[b_].T)
        in_maps.append(m)

    global LAST_IN_MAPS
    LAST_IN_MAPS = in_maps
    nc = _get_nc()
    res = run_bass_kernel_spmd(nc, in_maps, core_ids=list(range(B)), trace=TRACE)
    LAST_RESULTS = res
    out = np.stack([np.asarray(res.results[c]["logits"], np.float32) for c in range(B)])
    return out


# ---------------------------------------------------------------------------
# Timing helpers (test-only): replicate run_bass_via_pjrt's sharded jit with
# device-resident inputs so repeated calls measure (dispatch + NEFF exec).
# ---------------------------------------------------------------------------
LAST_IN_MAPS = None


def _sharded_exec(nc, in_maps):
    import jax
    from jax.experimental.shard_map import shard_map
    from jax.sharding import Mesh, PartitionSpec
    from concourse import bass2jax

    bass2jax.install_neuronx_cc_hook()
    n_cores = len(in_maps)
    partition_name = nc.partition_id_tensor.name if nc.partition_id_tensor else None
    in_names, out_names, out_avals, zero_outs = [], [], [], []
    for alloc in nc.m.functions[0].allocations:
        if not isinstance(alloc, mybir.MemoryLocationSet):
            continue
        name = alloc.memorylocations[0].name
        if alloc.kind == "ExternalInput":
            if name != partition_name:
                in_names.append(name)
        elif alloc.kind == "ExternalOutput":
            shape = tuple(alloc.tensor_shape)
            dtype = mybir.dt.np(alloc.dtype)
            out_names.append(name)
            out_avals.append(jax.core.ShapedArray(shape, dtype))
            zero_outs.append(np.zeros(shape, dtype))
    n_params = len(in_names)
    n_outs = len(out_avals)
    all_in_names = list(in_names) + list(out_names)
    if partition_name is not None:
        all_in_names.append(partition_name)
    donate = tuple(range(n_params, n_params + n_outs))

    def _body(*args):
        operands = list(args)
        if partition_name is not None:
            operands.append(bass2jax.partition_id_tensor())
        outs = bass2jax._bass_exec_p.bind(
            *operands,
            out_avals=tuple(out_avals),
            in_names=tuple(all_in_names),
            out_names=tuple(out_names),
            lowering_input_output_aliases=(),
            sim_require_finite=True,
            sim_require_nnan=True,
            nc=nc,
        )
        return tuple(outs)

    devices = jax.devices()[:n_cores]
    mesh = Mesh(np.asarray(devices), ("core",))
    sharded = jax.jit(
        shard_map(_body, mesh=mesh,
                  in_specs=(PartitionSpec("core"),) * (n_params + n_outs),
                  out_specs=(PartitionSpec("core"),) * n_outs,
                  check_rep=False),
        donate_argnums=donate, keep_unused=True)

    concat_in = [np.concatenate([np.asarray(m[name]) for m in in_maps], axis=0)
                 for name in in_names]
    concat_zeros = [np.zeros((n_cores * z.shape[0], *z.shape[1:]), z.dtype)
                    for z in zero_outs]
    from jax.sharding import NamedSharding
    sh = NamedSharding(mesh, PartitionSpec("core"))
    dev_in = [jax.device_put(a, sh) for a in concat_in]
    return sharded, dev_in, concat_zeros, sh


def _time_exec(nc, in_maps, iters):
    import time as _time
    import jax
    sharded, dev_in, concat_zeros, sh = _sharded_exec(nc, in_maps)
    times = []
    for _ in range(iters):
        zs = [jax.device_put(z, sh) for z in concat_zeros]
        jax.block_until_ready(zs)
        jax.block_until_ready(dev_in)
        t0 = _time.perf_counter()
        out = sharded(*dev_in, *zs)
        jax.block_until_ready(out)
        times.append(_time.perf_counter() - t0)
    return times


def timed_run(iters=5):
    assert LAST_IN_MAPS is not None, "call kernel() first"
    return _time_exec(_get_nc(), LAST_IN_MAPS, iters)


def timed_slope(ns=(1, 4, 12), zsets=12):
    """Async-dispatch n calls back-to-back; slope of total-time vs n ~ exec."""
    import time as _time
    import jax
    assert LAST_IN_MAPS is not None
    sharded, dev_in, concat_zeros, sh = _sharded_exec(_get_nc(), LAST_IN_MAPS)
    all_zs = [[jax.device_put(z, sh) for z in concat_zeros] for _ in range(zsets)]
    jax.block_until_ready(all_zs)
    jax.block_until_ready(dev_in)
    # warm
    out = sharded(*dev_in, *all_zs[0])
    jax.block_until_ready(out)
    res = {}
    for n in ns:
        zs_fresh = [[jax.device_put(z, sh) for z in concat_zeros] for _ in range(n)]
        jax.block_until_ready(zs_fresh)
        t0 = _time.perf_counter()
        outs = [sharded(*dev_in, *zs_fresh[i]) for i in range(n)]
        jax.block_until_ready(outs)
        res[n] = _time.perf_counter() - t0
    return res


def timed_repeat(r=5, iters=6):
    """exec_ns ~= (min_time(R=r NEFF) - min_time(R=1 NEFF)) / (r-1)."""
    assert LAST_IN_MAPS is not None
    t1 = min(_time_exec(_get_nc(1), LAST_IN_MAPS, iters))
    tr = min(_time_exec(_get_nc(r), LAST_IN_MAPS, iters))
    return (tr - t1) / (r - 1), t1, tr


# revision 11
# speedup vs baseline: 1.0325x; 1.0325x over previous
"""GPT forward kernel for 8 TRN2 NeuronCores (v2).

Data-parallel over batch (B=8 -> 1 sequence per core). Host folds the LN
affine params into the adjacent weights (exact), pre-transposes weights to
put the contraction dim on SBUF partitions, casts them to bf16, and does
the embedding gather. On device the residual stream is kept transposed
(x^T [D, T] fp32 in SBUF); LayerNorm is pure normalization whose stats
matmuls are interleaved into the producing GEMM, broadcast planes are
built on GpSimd, and PSUM drains ride the Scalar engine so the Tensor
engine stays dense.
"""
import sys
sys.path.insert(0, '/opt/trn_rl_repo')
import numpy as np
import ml_dtypes

import concourse.bass as bass
import concourse.tile as tile
from concourse import bacc, mybir
from concourse.bass_utils import run_bass_kernel_spmd

B, T, D, H, L, V, MAXT = 8, 1024, 1024, 16, 8, 8192, 4096
HD = D // H          # 64
P = 128
DS = D // P          # 8 d-subtiles
TS = T // P          # 8 t-subtiles
D2S = (2 * D) // P   # 16 mlp subtiles
VS = V // 512        # 16 vocab chunks
NCH = 512
EPS = 1e-5
SCALE = 1.0 / np.sqrt(HD)

F32 = mybir.dt.float32
BF16 = mybir.dt.bfloat16
AF = mybir.ActivationFunctionType
ALU = mybir.AluOpType

# smalls[:, col] layout, per layer base = l*32  (bo, b2: [P,1]-packed cols)
SM_BO, SM_B2, SM_B1 = 0, 8, 16
SM_PER_LAYER = 32
SM_COLS = L * SM_PER_LAYER

TRACE = False
LAST_RESULTS = None


def _build(repeat=1):
    import contextlib
    nc = bacc.Bacc("TRN2", target_bir_lowering=False)

    x0T_d = nc.dram_tensor("x0T", [D, T], F32, kind="ExternalInput")
    WqT_d = nc.dram_tensor("WqT", [L, D, D], BF16, kind="ExternalInput")
    WkT_d = nc.dram_tensor("WkT", [L, D, D], BF16, kind="ExternalInput")
    WvT_d = nc.dram_tensor("WvT", [L, D, D], BF16, kind="ExternalInput")
    WoT_d = nc.dram_tensor("WoT", [L, D, D], BF16, kind="ExternalInput")
    W1T_d = nc.dram_tensor("W1T", [L, D, 2 * D], BF16, kind="ExternalInput")
    W2T_d = nc.dram_tensor("W2T", [L, 2 * D, D], BF16, kind="ExternalInput")
    hT_w_d = nc.dram_tensor("headT", [D, V], BF16, kind="ExternalInput")
    sm_d = nc.dram_tensor("smalls", [P, SM_COLS], F32, kind="ExternalInput")
    brow_d = nc.dram_tensor("brows", [L, 2 * D], BF16, kind="ExternalInput")
    bv_d = nc.dram_tensor("bvB", [1, L * D], BF16, kind="ExternalInput")
    hb_d = nc.dram_tensor("hbB", [1, V], BF16, kind="ExternalInput")
    mask_d = nc.dram_tensor("mask01", [P, P], BF16, kind="ExternalInput")
    out_d = nc.dram_tensor("logits", [T, V], F32, kind="ExternalOutput")

    out_r = out_d[:, :].rearrange("(t pi) v -> pi t v", pi=P)

    with tile.TileContext(nc) as tc:
        with (
            tc.tile_pool(name="pc", bufs=1) as pc,
            tc.tile_pool(name="pw", bufs=3) as pw,
            tc.tile_pool(name="pbv", bufs=2) as pbv,
            tc.tile_pool(name="pbr", bufs=2) as pbr,
            tc.tile_pool(name="phb", bufs=2) as phb,
            tc.tile_pool(name="ppt", bufs=2) as ppt,
            tc.tile_pool(name="px", bufs=3) as px,
            tc.tile_pool(name="pr", bufs=3) as pr,
            tc.tile_pool(name="pln", bufs=4) as pln,
            tc.tile_pool(name="pbc", bufs=2) as pbc,
            tc.tile_pool(name="pps", bufs=4, space="PSUM") as pps,
            tc.tile_pool(name="ppo", bufs=2, space="PSUM") as ppo,
            tc.tile_pool(name="pst", bufs=2, space="PSUM") as pst,
        ):
            xT = pc.tile([P, DS, T], F32)     # residual, transposed
            xb = pc.tile([P, DS, T], BF16)    # bf16 copy of residual (LN input)
            hT = pc.tile([P, DS, T], BF16)    # LN output; reused as attn y^T
            qkT = pc.tile([P, 2 * DS, T], BF16)  # q rows 0:8, k rows 8:16; reused as gT
            Vg = pc.tile([P, TS, H, HD + 1], BF16)
            yT = hT
            sm = pc.tile([P, SM_COLS], F32)
            mask = pc.tile([P, P], BF16)
            ones_row = pc.tile([1, NCH], BF16)
            ones_cb = pc.tile([P, 1], BF16)
            eps_t = pc.tile([1, 1], F32)

            nc.vector.memset(ones_row[:], 1.0)
            nc.vector.memset(ones_cb[:], 1.0)
            nc.vector.memset(eps_t[:], EPS)
            nc.vector.memset(Vg[:, :, :, HD:HD + 1], 1.0)
            nc.sync.dma_start(sm[:], sm_d[:, :])
            nc.sync.dma_start(mask[:], mask_d[:, :])

            def ln_stats_k(stat2c, c, k):
                """stat2c: PSUM [65, NCH]; row 0 accumulates sum, row 64 sumsq."""
                tch = bass.ts(c, NCH)
                nc.vector.tensor_copy(xb[:, k, tch], xT[:, k, tch])
                sq = px.tile([P, NCH], BF16, tag="sq")
                nc.scalar.activation(sq[:], xb[:, k, tch], AF.Square)
                nc.tensor.matmul(stat2c[0:1, :], ones_cb[:],
                                 xb[:, k, tch], start=(k == 0),
                                 stop=(k == DS - 1), skip_group_check=True)
                nc.tensor.matmul(stat2c[64:65, :], ones_cb[:],
                                 sq[:], start=(k == 0), stop=(k == DS - 1),
                                 skip_group_check=True)

            def ln_finish(stat2c):
                """-> (A, B) bf16 [P, NCH] planes: h = x*A + B."""
                m2 = pr.tile([1, NCH], F32, tag="r")
                nc.scalar.activation(m2[:], stat2c[0:1, :],
                                     AF.Square, scale=1.0 / D)
                var = pr.tile([1, NCH], F32, tag="r")
                nc.vector.scalar_tensor_tensor(var[:], stat2c[64:65, :],
                                               1.0 / D, m2[:],
                                               op0=ALU.mult, op1=ALU.subtract)
                sd = pr.tile([1, NCH], F32, tag="r")
                nc.scalar.activation(sd[:], var[:], AF.Sqrt, bias=eps_t[:])
                srow = pr.tile([1, NCH], BF16, tag="rb")
                nm = pr.tile([1, NCH], BF16, tag="rb")
                with nc.allow_low_precision(reason="LN planes applied in bf16 anyway"):
                    nc.vector.reciprocal(srow[:], sd[:])
                    nc.vector.scalar_tensor_tensor(nm[:], stat2c[0:1, :],
                                                   -1.0 / D, srow[:],
                                                   op0=ALU.mult, op1=ALU.mult)
                A = pln.tile([P, NCH], BF16, tag="pl")
                nc.gpsimd.partition_broadcast(A[:], srow[:], channels=P)
                Bp = pln.tile([P, NCH], BF16, tag="pl")
                nc.gpsimd.partition_broadcast(Bp[:], nm[:], channels=P)
                return A, Bp

            def ln_apply(A, Bp, c):
                tch = bass.ts(c, NCH)
                for k in range(DS):
                    tmp = px.tile([P, NCH], BF16, tag="lt")
                    nc.vector.tensor_mul(tmp[:], xb[:, k, tch], A[:])
                    nc.vector.tensor_add(hT[:, k, tch], tmp[:], Bp[:])

            loop_cm = tc.For_i(0, repeat, 1) if repeat > 1 else contextlib.nullcontext()
            with loop_cm:
                nc.sync.dma_start(xT[:], x0T_d[:, :].rearrange("(po pi) t -> pi po t", pi=P))
                stat0 = pst.tile([65, NCH], F32, tag="st")
                stat1 = pst.tile([65, NCH], F32, tag="st")
                stat = [stat0, stat1]
                for c in range(2):
                    for k in range(DS):
                        ln_stats_k(stat[c], c, k)
                    A0c, B0c = ln_finish(stat[c])
                    ln_apply(A0c, B0c, c)

                for l in range(L):
                    base = l * SM_PER_LAYER
                    br_t = pbr.tile([1, 2 * D], BF16, tag="br")
                    nc.sync.dma_start(br_t[:], brow_d[l:l + 1, :])

                    # ---- q^T / k^T projections (bias via K=1 matmul row) ----
                    for which, W_d in ((0, WqT_d), (1, WkT_d)):
                        qoff = which * DS
                        for half in range(2):
                            wsl = pw.tile([P, DS, NCH], BF16, tag="w")
                            nc.sync.dma_start(
                                wsl[:],
                                W_d[l].rearrange("(po pi) o -> pi po o", pi=P)[:, :, bass.ts(half, NCH)])
                            for m in range(4):
                                mo = half * 4 + m
                                bsl = br_t[0:1, which * D + mo * P: which * D + (mo + 1) * P]
                                ps0 = pps.tile([P, NCH], F32, tag="a")
                                ps1 = pps.tile([P, NCH], F32, tag="a")
                                nc.tensor.matmul(ps0[:], bsl, ones_row[:], start=True, stop=False)
                                nc.tensor.matmul(ps1[:], bsl, ones_row[:], start=True, stop=False)
                                for k in range(DS):
                                    nc.tensor.matmul(ps0[:], wsl[:, k, bass.ts(m, P)],
                                                     hT[:, k, bass.ts(0, NCH)],
                                                     start=False, stop=(k == DS - 1))
                                    nc.tensor.matmul(ps1[:], wsl[:, k, bass.ts(m, P)],
                                                     hT[:, k, bass.ts(1, NCH)],
                                                     start=False, stop=(k == DS - 1))
                                nc.scalar.activation(qkT[:, qoff + mo, bass.ts(0, NCH)], ps0[:], AF.Copy)
                                nc.scalar.activation(qkT[:, qoff + mo, bass.ts(1, NCH)], ps1[:], AF.Copy)

                    # ---- V projection: out[t, o] into Vg ----
                    for half in range(2):
                        wsl = pw.tile([P, DS, NCH], BF16, tag="w")
                        nc.sync.dma_start(
                            wsl[:],
                            WvT_d[l].rearrange("(po pi) o -> pi po o", pi=P)[:, :, bass.ts(half, NCH)])
                        bvs = pbv.tile([1, NCH], BF16, tag="bvs")
                        nc.sync.dma_start(bvs[:], bv_d[:, l * D + half * NCH:l * D + (half + 1) * NCH])
                        for t_ in range(TS):
                            ps_t = pps.tile([P, NCH], F32, tag="a")
                            nc.tensor.matmul(ps_t[:], ones_row[0:1, 0:P], bvs[:],
                                             start=True, stop=False)
                            for k in range(DS):
                                nc.tensor.matmul(ps_t[:], hT[:, k, bass.ts(t_, P)],
                                                 wsl[:, k, :],
                                                 start=False, stop=(k == DS - 1))
                            nc.scalar.activation(
                                Vg[:, t_, 8 * half:8 * half + 8, 0:HD],
                                ps_t[:].rearrange("p (h d) -> p h d", d=HD), AF.Copy)

                    # ---- attention: scores(u) pipelined over PV/norm(u-1) ----
                    def attn_scores(h, c):
                        pbase = (h % 2) * HD
                        sub = h // 2
                        PT = ppt.tile([P, TS, NCH], BF16, tag="pt")
                        ntk = 4 * c + 4
                        for tk in range(ntk):
                            ls = max(0, tk * P - c * NCH)
                            w_ = NCH - ls
                            sT = pps.tile([P, NCH], F32, tag="a")
                            nc.tensor.matmul(
                                sT[:, :w_],
                                qkT[pbase:pbase + HD, DS + sub, bass.ts(tk, P)],
                                qkT[pbase:pbase + HD, sub, c * NCH + ls:(c + 1) * NCH],
                                start=True, stop=True)
                            nc.scalar.activation(PT[:, tk, ls:], sT[:, :w_], AF.Exp,
                                                 scale=float(SCALE))
                            if tk >= 4 * c:
                                nc.vector.tensor_mul(PT[:, tk, ls:ls + P],
                                                     PT[:, tk, ls:ls + P], mask[:])
                        return PT

                    def attn_pv(h, c, PT):
                        pbase = (h % 2) * HD
                        sub = h // 2
                        tch = bass.ts(c, NCH)
                        ntk = 4 * c + 4
                        po = ppo.tile([HD + 1, NCH], F32, tag="o")
                        for tk in range(ntk):
                            ls = max(0, tk * P - c * NCH)
                            nc.tensor.matmul(po[:, ls:], Vg[:, tk, h, :],
                                             PT[:, tk, ls:],
                                             start=(tk == 0), stop=(tk == ntk - 1))
                        dn = pr.tile([1, NCH], F32, tag="dn")
                        nc.vector.reciprocal(dn[:], po[HD:HD + 1, :])
                        bc = pbc.tile([HD, NCH], F32, tag="bc")
                        nc.gpsimd.partition_broadcast(bc[:], dn[:], channels=HD)
                        nc.vector.tensor_mul(yT[pbase:pbase + HD, sub, tch],
                                             po[0:HD, :], bc[:])

                    prev = None
                    for h in range(H):
                        for c in range(2):
                            PT = attn_scores(h, c)
                            if prev is not None:
                                attn_pv(*prev)
                            prev = (h, c, PT)
                    attn_pv(*prev)

                    # ---- attention out projection + residual + LN2 stats ----
                    wo0 = pw.tile([P, DS, NCH], BF16, tag="w")
                    nc.sync.dma_start(wo0[:], WoT_d[l].rearrange("(po pi) o -> pi po o", pi=P)[:, :, bass.ts(0, NCH)])
                    wo1 = pw.tile([P, DS, NCH], BF16, tag="w")
                    nc.sync.dma_start(wo1[:], WoT_d[l].rearrange("(po pi) o -> pi po o", pi=P)[:, :, bass.ts(1, NCH)])
                    stat2a = pst.tile([65, NCH], F32, tag="st")
                    stat2b = pst.tile([65, NCH], F32, tag="st")
                    stat2 = [stat2a, stat2b]
                    for c in range(2):
                        tch = bass.ts(c, NCH)
                        for mo in range(DS):
                            wsl_o = wo0 if mo < 4 else wo1
                            m = mo % 4
                            ps0 = pps.tile([P, NCH], F32, tag="a")
                            for k in range(DS):
                                nc.tensor.matmul(ps0[:], wsl_o[:, k, bass.ts(m, P)],
                                                 yT[:, k, tch],
                                                 start=(k == 0), stop=(k == DS - 1))
                            nc.vector.scalar_tensor_tensor(
                                xT[:, mo, tch], ps0[:],
                                sm[:, base + SM_BO + mo:base + SM_BO + mo + 1],
                                xT[:, mo, tch], op0=ALU.add, op1=ALU.add)
                            ln_stats_k(stat2[c], c, mo)
                        A2c, B2c = ln_finish(stat2[c])
                        ln_apply(A2c, B2c, c)


                    # ---- MLP W1 -> gelu -> gT (qkT reused) ----
                    for quarter in range(4):
                        wsl = pw.tile([P, DS, NCH], BF16, tag="w")
                        nc.sync.dma_start(
                            wsl[:],
                            W1T_d[l].rearrange("(po pi) o -> pi po o", pi=P)[:, :, bass.ts(quarter, NCH)])
                        for m in range(4):
                            mo = quarter * 4 + m
                            ps0 = pps.tile([P, NCH], F32, tag="a")
                            ps1 = pps.tile([P, NCH], F32, tag="a")
                            for k in range(DS):
                                nc.tensor.matmul(ps0[:], wsl[:, k, bass.ts(m, P)],
                                                 hT[:, k, bass.ts(0, NCH)],
                                                 start=(k == 0), stop=(k == DS - 1))
                                nc.tensor.matmul(ps1[:], wsl[:, k, bass.ts(m, P)],
                                                 hT[:, k, bass.ts(1, NCH)],
                                                 start=(k == 0), stop=(k == DS - 1))
                            for c, psx in ((0, ps0), (1, ps1)):
                                nc.scalar.activation(
                                    qkT[:, mo, bass.ts(c, NCH)], psx[:], AF.Gelu,
                                    bias=sm[:, base + SM_B1 + mo:base + SM_B1 + mo + 1])

                    # ---- W2 + residual + next-LN stats ----
                    stat3a = pst.tile([65, NCH], F32, tag="st")
                    stat3b = pst.tile([65, NCH], F32, tag="st")
                    stat3 = [stat3a, stat3b]
                    for c in range(2):
                        tch = bass.ts(c, NCH)
                        for quarter in range(4):
                            w2q = pw.tile([P, D2S, P * 2], BF16, tag="w")
                            nc.sync.dma_start(
                                w2q[:],
                                W2T_d[l].rearrange("(po pi) o -> pi po o", pi=P)[:, :, bass.ts(quarter, P * 2)])
                            for m in range(2):
                                mo = quarter * 2 + m
                                ps0 = pps.tile([P, NCH], F32, tag="a")
                                for k in range(D2S):
                                    nc.tensor.matmul(ps0[:], w2q[:, k, bass.ts(m, P)],
                                                     qkT[:, k, tch],
                                                     start=(k == 0), stop=(k == D2S - 1))
                                nc.vector.scalar_tensor_tensor(
                                    xT[:, mo, tch], ps0[:],
                                    sm[:, base + SM_B2 + mo:base + SM_B2 + mo + 1],
                                    xT[:, mo, tch], op0=ALU.add, op1=ALU.add)
                                ln_stats_k(stat3[c], c, mo)
                        A3c, B3c = ln_finish(stat3[c])
                        ln_apply(A3c, B3c, c)

                # ---- vocab head (final LN already applied in last W2 pass) ----
                hw_r = hT_w_d[:, :].rearrange("(po pi) v -> pi po v", pi=P)
                for vp in range(VS // 2):
                    ws0 = pw.tile([P, DS, NCH], BF16, tag="w")
                    nc.sync.dma_start(ws0[:], hw_r[:, :, bass.ts(2 * vp, NCH)])
                    ws1 = pw.tile([P, DS, NCH], BF16, tag="w")
                    nc.sync.dma_start(ws1[:], hw_r[:, :, bass.ts(2 * vp + 1, NCH)])
                    hb2 = phb.tile([1, 2 * NCH], BF16, tag="hb")
                    nc.sync.dma_start(hb2[:], hb_d[:, 2 * vp * NCH:(2 * vp + 2) * NCH])
                    for t_ in range(TS):
                        ps0 = pps.tile([P, NCH], F32, tag="a")
                        ps1 = pps.tile([P, NCH], F32, tag="a")
                        nc.tensor.matmul(ps0[:], ones_row[0:1, 0:P], hb2[0:1, 0:NCH],
                                         start=True, stop=False)
                        nc.tensor.matmul(ps1[:], ones_row[0:1, 0:P], hb2[0:1, NCH:2 * NCH],
                                         start=True, stop=False)
                        for k in range(DS):
                            nc.tensor.matmul(ps0[:], hT[:, k, bass.ts(t_, P)],
                                             ws0[:, k, :],
                                             start=False, stop=(k == DS - 1))
                            nc.tensor.matmul(ps1[:], hT[:, k, bass.ts(t_, P)],
                                             ws1[:, k, :],
                                             start=False, stop=(k == DS - 1))
                        for j, psx in ((0, ps0), (1, ps1)):
                            ot = px.tile([P, NCH], F32, tag="ot")
                            nc.scalar.activation(ot[:], psx[:], AF.Copy)
                            nc.sync.dma_start(out_r[:, t_, bass.ts(2 * vp + j, NCH)], ot[:])

    nc.compile()
    return nc


_NC = {}


def _get_nc(repeat=1):
    if repeat not in _NC:
        _NC[repeat] = _build(repeat)
    return _NC[repeat]


def _pack_cols(vec, ncols):
    """[ncols*128] -> [128, ncols] with column j = vec[j*128:(j+1)*128]."""
    return np.ascontiguousarray(vec.reshape(ncols, P).T)


def kernel(idx, timesteps, tok_emb_w, pos_emb, global_pos_emb,
           ln1_w, ln1_b, Wq, bq, Wk, bk, Wv, bv, Wo, bo,
           ln2_w, ln2_b, W1, b1, W2, b2, lnf_w, lnf_b, head_w):
    global LAST_RESULTS
    f = lambda a: np.asarray(a, dtype=np.float32)
    idx = np.asarray(idx, dtype=np.int64)
    tsteps = np.asarray(timesteps, dtype=np.int64)
    tok_emb_w, pos_emb, global_pos_emb = f(tok_emb_w), f(pos_emb), f(global_pos_emb)

    # embedding on host (tiny compute, avoids on-device gather)
    x0 = tok_emb_w[idx] + global_pos_emb[0][tsteps[:, 0]][:, None, :] + pos_emb[:, :T]
    x0 = np.ascontiguousarray(x0.astype(np.float32))

    # fold LN affine params into adjacent weights (exact rewrite)
    Wq, bq, Wk, bk, Wv, bv = f(Wq), f(bq), f(Wk), f(bk), f(Wv), f(bv)
    Wo, bo, W1, b1, W2, b2 = f(Wo), f(bo), f(W1), f(b1), f(W2), f(b2)
    ln1_w, ln1_b, ln2_w, ln2_b = f(ln1_w), f(ln1_b), f(ln2_w), f(ln2_b)
    lnf_w, lnf_b, head_w = f(lnf_w), f(lnf_b), f(head_w)

    Wqf = Wq * ln1_w[:, None, :]
    bqf = bq + np.einsum('lod,ld->lo', Wq, ln1_b)
    Wkf = Wk * ln1_w[:, None, :]
    bkf = bk + np.einsum('lod,ld->lo', Wk, ln1_b)
    Wvf = Wv * ln1_w[:, None, :]
    bvf = bv + np.einsum('lod,ld->lo', Wv, ln1_b)
    W1f = W1 * ln2_w[:, None, :]
    b1f = b1 + np.einsum('lod,ld->lo', W1, ln2_b)
    headf = head_w * lnf_w[None, :]
    hb = head_w @ lnf_b

    bf = lambda a: np.ascontiguousarray(np.asarray(a, np.float32)).astype(ml_dtypes.bfloat16)
    shared = {
        "WqT": bf(Wqf.transpose(0, 2, 1)),
        "WkT": bf(Wkf.transpose(0, 2, 1)),
        "WvT": bf(Wvf.transpose(0, 2, 1)),
        "WoT": bf(Wo.transpose(0, 2, 1)),
        "W1T": bf(W1f.transpose(0, 2, 1)),
        "W2T": bf(W2.transpose(0, 2, 1)),
        "headT": bf(headf.T),
        "bvB": bf(bvf.reshape(1, L * D)),
        "hbB": bf(hb.reshape(1, V)),
        "brows": bf(np.concatenate([bqf, bkf], axis=1)),  # [L, 2D]
    }
    smalls = np.zeros((P, SM_COLS), np.float32)
    for l in range(L):
        b_ = l * SM_PER_LAYER
        smalls[:, b_ + SM_BO:b_ + SM_BO + 8] = _pack_cols(bo[l], DS)
        smalls[:, b_ + SM_B2:b_ + SM_B2 + 8] = _pack_cols(b2[l], DS)
        smalls[:, b_ + SM_B1:b_ + SM_B1 + 16] = _pack_cols(b1f[l], D2S)
    shared["smalls"] = smalls

    m01 = (np.arange(P)[:, None] <= np.arange(P)[None, :])
    shared["mask01"] = m01.astype(ml_dtypes.bfloat16)

    in_maps = []
    for b_ in range(B):
        m = dict(shared)
        m["x0T"] = np.ascontiguousarray(x0[b_].T)
        in_maps.append(m)

    global LAST_IN_MAPS
    LAST_IN_MAPS = in_maps
    nc = _get_nc()
    res = run_bass_kernel_spmd(nc, in_maps, core_ids=list(range(B)), trace=TRACE)
    LAST_RESULTS = res
    out = np.stack([np.asarray(res.results[c]["logits"], np.float32) for c in range(B)])
    return out


# ---------------------------------------------------------------------------
# Timing helpers (test-only): replicate run_bass_via_pjrt's sharded jit with
# device-resident inputs so repeated calls measure (dispatch + NEFF exec).
# ---------------------------------------------------------------------------
LAST_IN_MAPS = None


def _sharded_exec(nc, in_maps):
    import jax
    from jax.experimental.shard_map import shard_map
    from jax.sharding import Mesh, PartitionSpec
    from concourse import bass2jax

    bass2jax.install_neuronx_cc_hook()
    n_cores = len(in_maps)
    partition_name = nc.partition_id_tensor.name if nc.partition_id_tensor else None
    in_names, out_names, out_avals, zero_outs = [], [], [], []
    for alloc in nc.m.functions[0].allocations:
        if not isinstance(alloc, mybir.MemoryLocationSet):
            continue
        name = alloc.memorylocations[0].name
        if alloc.kind == "ExternalInput":
            if name != partition_name:
                in_names.append(name)
        elif alloc.kind == "ExternalOutput":
            shape = tuple(alloc.tensor_shape)
            dtype = mybir.dt.np(alloc.dtype)
            out_names.append(name)
            out_avals.append(jax.core.ShapedArray(shape, dtype))
            zero_outs.append(np.zeros(shape, dtype))
    n_params = len(in_names)
    n_outs = len(out_avals)
    all_in_names = list(in_names) + list(out_names)
    if partition_name is not None:
        all_in_names.append(partition_name)
    donate = tuple(range(n_params, n_params + n_outs))

    def _body(*args):
        operands = list(args)
        if partition_name is not None:
            operands.append(bass2jax.partition_id_tensor())
        outs = bass2jax._bass_exec_p.bind(
            *operands,
            out_avals=tuple(out_avals),
            in_names=tuple(all_in_names),
            out_names=tuple(out_names),
            lowering_input_output_aliases=(),
            sim_require_finite=True,
            sim_require_nnan=True,
            nc=nc,
        )
        return tuple(outs)

    devices = jax.devices()[:n_cores]
    mesh = Mesh(np.asarray(devices), ("core",))
    sharded = jax.jit(
        shard_map(_body, mesh=mesh,
                  in_specs=(PartitionSpec("core"),) * (n_params + n_outs),
                  out_specs=(PartitionSpec("core"),) * n_outs,
                  check_rep=False),
        donate_argnums=donate, keep_unused=True)

    concat_in = [np.concatenate([np.asarray(m[name]) for m in in_maps], axis=0)
                 for name in in_names]
    concat_zeros = [np.zeros((n_cores * z.shape[0], *z.shape[1:]), z.dtype)
                    for z in zero_outs]
    from jax.sharding import NamedSharding
    sh = NamedSharding(mesh, PartitionSpec("core"))
    dev_in = [jax.device_put(a, sh) for a in concat_in]
    return sharded, dev_in, concat_zeros, sh


def _time_exec(nc, in_maps, iters):
    import time as _time
    import jax
    sharded, dev_in, concat_zeros, sh = _sharded_exec(nc, in_maps)
    times = []
    for _ in range(iters):
        zs = [jax.device_put(z, sh) for z in concat_zeros]
        jax.block_until_ready(zs)
        jax.block_until_ready(dev_in)
        t0 = _time.perf_counter()
        out = sharded(*dev_in, *zs)
        jax.block_until_ready(out)
        times.append(_time.perf_counter() - t0)
    return times


def timed_run(iters=5):
    assert LAST_IN_MAPS is not None, "call kernel() first"
    return _time_exec(_get_nc(), LAST_IN_MAPS, iters)


def timed_slope(ns=(1, 4, 12), zsets=12):
    """Async-dispatch n calls back-to-back; slope of total-time vs n ~ exec."""
    import time as _time
    import jax
    assert LAST_IN_MAPS is not None
    sharded, dev_in, concat_zeros, sh = _sharded_exec(_get_nc(), LAST_IN_MAPS)
    all_zs = [[jax.device_put(z, sh) for z in concat_zeros] for _ in range(zsets)]
    jax.block_until_ready(all_zs)
    jax.block_until_ready(dev_in)
    # warm
    out = sharded(*dev_in, *all_zs[0])
    jax.block_until_ready(out)
    res = {}
    for n in ns:
        zs_fresh = [[jax.device_put(z, sh) for z in concat_zeros] for _ in range(n)]
        jax.block_until_ready(zs_fresh)
        t0 = _time.perf_counter()
        outs = [sharded(*dev_in, *zs_fresh[i]) for i in range(n)]
        jax.block_until_ready(outs)
        res[n] = _time.perf_counter() - t0
    return res


def timed_repeat(r=5, iters=6):
    """exec_ns ~= (min_time(R=r NEFF) - min_time(R=1 NEFF)) / (r-1)."""
    assert LAST_IN_MAPS is not None
    t1 = min(_time_exec(_get_nc(1), LAST_IN_MAPS, iters))
    tr = min(_time_exec(_get_nc(r), LAST_IN_MAPS, iters))
    return (tr - t1) / (r - 1), t1, tr


# revision 12
# speedup vs baseline: 1.0526x; 1.0194x over previous
"""GPT forward kernel for 8 TRN2 NeuronCores (v2).

Data-parallel over batch (B=8 -> 1 sequence per core). Host folds the LN
affine params into the adjacent weights (exact), pre-transposes weights to
put the contraction dim on SBUF partitions, casts them to bf16, and does
the embedding gather. On device the residual stream is kept transposed
(x^T [D, T] fp32 in SBUF); LayerNorm is pure normalization whose stats
matmuls are interleaved into the producing GEMM, broadcast planes are
built on GpSimd, and PSUM drains ride the Scalar engine so the Tensor
engine stays dense.
"""
import sys
sys.path.insert(0, '/opt/trn_rl_repo')
import numpy as np
import ml_dtypes

import concourse.bass as bass
import concourse.tile as tile
from concourse import bacc, mybir
from concourse.bass_utils import run_bass_kernel_spmd

B, T, D, H, L, V, MAXT = 8, 1024, 1024, 16, 8, 8192, 4096
HD = D // H          # 64
P = 128
DS = D // P          # 8 d-subtiles
TS = T // P          # 8 t-subtiles
D2S = (2 * D) // P   # 16 mlp subtiles
VS = V // 512        # 16 vocab chunks
NCH = 512
EPS = 1e-5
SCALE = 1.0 / np.sqrt(HD)

F32 = mybir.dt.float32
BF16 = mybir.dt.bfloat16
AF = mybir.ActivationFunctionType
ALU = mybir.AluOpType

# smalls[:, col] layout, per layer base = l*32  (bo, b2: [P,1]-packed cols)
SM_BO, SM_B2, SM_B1 = 0, 8, 16
SM_PER_LAYER = 32
SM_COLS = L * SM_PER_LAYER

TRACE = False
LAST_RESULTS = None


def _build(repeat=1):
    import contextlib
    nc = bacc.Bacc("TRN2", target_bir_lowering=False)

    x0T_d = nc.dram_tensor("x0T", [D, T], F32, kind="ExternalInput")
    WqT_d = nc.dram_tensor("WqT", [L, D, D], BF16, kind="ExternalInput")
    WkT_d = nc.dram_tensor("WkT", [L, D, D], BF16, kind="ExternalInput")
    WvT_d = nc.dram_tensor("WvT", [L, D, D], BF16, kind="ExternalInput")
    WoT_d = nc.dram_tensor("WoT", [L, D, D], BF16, kind="ExternalInput")
    W1T_d = nc.dram_tensor("W1T", [L, D, 2 * D], BF16, kind="ExternalInput")
    W2T_d = nc.dram_tensor("W2T", [L, 2 * D, D], BF16, kind="ExternalInput")
    hT_w_d = nc.dram_tensor("headT", [D, V], BF16, kind="ExternalInput")
    sm_d = nc.dram_tensor("smalls", [P, SM_COLS], F32, kind="ExternalInput")
    brow_d = nc.dram_tensor("brows", [L, 2 * D], BF16, kind="ExternalInput")
    bv_d = nc.dram_tensor("bvB", [1, L * D], BF16, kind="ExternalInput")
    hb_d = nc.dram_tensor("hbB", [1, V], BF16, kind="ExternalInput")
    mask_d = nc.dram_tensor("mask01", [P, P], BF16, kind="ExternalInput")
    out_d = nc.dram_tensor("logits", [T, V], F32, kind="ExternalOutput")

    out_r = out_d[:, :].rearrange("(t pi) v -> pi t v", pi=P)

    with tile.TileContext(nc) as tc:
        with (
            tc.tile_pool(name="pc", bufs=1) as pc,
            tc.tile_pool(name="pw", bufs=3) as pw,
            tc.tile_pool(name="pbv", bufs=2) as pbv,
            tc.tile_pool(name="pbr", bufs=2) as pbr,
            tc.tile_pool(name="phb", bufs=2) as phb,
            tc.tile_pool(name="ppt", bufs=2) as ppt,
            tc.tile_pool(name="px", bufs=3) as px,
            tc.tile_pool(name="pr", bufs=3) as pr,
            tc.tile_pool(name="pln", bufs=4) as pln,
            tc.tile_pool(name="pbc", bufs=2) as pbc,
            tc.tile_pool(name="pps", bufs=4, space="PSUM") as pps,
            tc.tile_pool(name="ppo", bufs=2, space="PSUM") as ppo,
            tc.tile_pool(name="pst", bufs=2, space="PSUM") as pst,
        ):
            xT = pc.tile([P, DS, T], F32)     # residual, transposed
            xb = pc.tile([P, DS, T], BF16)    # bf16 copy of residual (LN input)
            hT = pc.tile([P, DS, T], BF16)    # LN output; reused as attn y^T
            qkT = pc.tile([P, 2 * DS, T], BF16)  # q rows 0:8, k rows 8:16; reused as gT
            Vg = pc.tile([P, TS, H, HD + 1], BF16)
            yT = hT
            sm = pc.tile([P, SM_COLS], F32)
            mask = pc.tile([P, P], BF16)
            ones_row = pc.tile([1, NCH], BF16)
            ones_cb = pc.tile([P, 1], BF16)
            eps_t = pc.tile([1, 1], F32)

            nc.vector.memset(ones_row[:], 1.0)
            nc.vector.memset(ones_cb[:], 1.0)
            nc.vector.memset(eps_t[:], EPS)
            nc.vector.memset(Vg[:, :, :, HD:HD + 1], 1.0)
            nc.sync.dma_start(sm[:], sm_d[:, :])
            nc.sync.dma_start(mask[:], mask_d[:, :])

            def ln_stats_k(stat2c, c, k):
                """stat2c: PSUM [65, NCH]; row 0 accumulates sum, row 64 sumsq."""
                tch = bass.ts(c, NCH)
                nc.vector.tensor_copy(xb[:, k, tch], xT[:, k, tch])
                sq = px.tile([P, NCH], BF16, tag="sq")
                nc.scalar.activation(sq[:], xb[:, k, tch], AF.Square)
                nc.tensor.matmul(stat2c[0:1, :], ones_cb[:],
                                 xb[:, k, tch], start=(k == 0),
                                 stop=(k == DS - 1), skip_group_check=True)
                nc.tensor.matmul(stat2c[64:65, :], ones_cb[:],
                                 sq[:], start=(k == 0), stop=(k == DS - 1),
                                 skip_group_check=True)

            def ln_finish(stat2c):
                """-> (A, B) bf16 [P, NCH] planes: h = x*A + B."""
                m2 = pr.tile([1, NCH], F32, tag="r")
                nc.scalar.activation(m2[:], stat2c[0:1, :],
                                     AF.Square, scale=1.0 / D)
                var = pr.tile([1, NCH], F32, tag="r")
                nc.vector.scalar_tensor_tensor(var[:], stat2c[64:65, :],
                                               1.0 / D, m2[:],
                                               op0=ALU.mult, op1=ALU.subtract)
                sd = pr.tile([1, NCH], F32, tag="r")
                nc.scalar.activation(sd[:], var[:], AF.Sqrt, bias=eps_t[:])
                srow = pr.tile([1, NCH], BF16, tag="rb")
                nm = pr.tile([1, NCH], BF16, tag="rb")
                with nc.allow_low_precision(reason="LN planes applied in bf16 anyway"):
                    nc.vector.reciprocal(srow[:], sd[:])
                    nc.vector.scalar_tensor_tensor(nm[:], stat2c[0:1, :],
                                                   -1.0 / D, srow[:],
                                                   op0=ALU.mult, op1=ALU.mult)
                A = pln.tile([P, NCH], BF16, tag="pl")
                nc.gpsimd.partition_broadcast(A[:], srow[:], channels=P)
                Bp = pln.tile([P, NCH], BF16, tag="pl")
                nc.gpsimd.partition_broadcast(Bp[:], nm[:], channels=P)
                return A, Bp

            def ln_apply(A, Bp, c):
                tch = bass.ts(c, NCH)
                for k in range(DS):
                    tmp = px.tile([P, NCH], BF16, tag="lt")
                    nc.vector.tensor_mul(tmp[:], xb[:, k, tch], A[:])
                    nc.vector.tensor_add(hT[:, k, tch], tmp[:], Bp[:])

            loop_cm = tc.For_i(0, repeat, 1) if repeat > 1 else contextlib.nullcontext()
            with loop_cm:
                nc.sync.dma_start(xT[:], x0T_d[:, :].rearrange("(po pi) t -> pi po t", pi=P))
                stat0 = pst.tile([65, NCH], F32, tag="st")
                stat1 = pst.tile([65, NCH], F32, tag="st")
                stat = [stat0, stat1]
                for c in range(2):
                    for k in range(DS):
                        ln_stats_k(stat[c], c, k)
                    A0c, B0c = ln_finish(stat[c])
                    ln_apply(A0c, B0c, c)

                for l in range(L):
                    base = l * SM_PER_LAYER
                    br_t = pbr.tile([1, 2 * D], BF16, tag="br")
                    nc.sync.dma_start(br_t[:], brow_d[l:l + 1, :])

                    # ---- q^T / k^T projections (bias via K=1 matmul row) ----
                    for which, W_d in ((0, WqT_d), (1, WkT_d)):
                        qoff = which * DS
                        for half in range(2):
                            wsl = pw.tile([P, DS, NCH], BF16, tag="w")
                            nc.sync.dma_start(
                                wsl[:],
                                W_d[l].rearrange("(po pi) o -> pi po o", pi=P)[:, :, bass.ts(half, NCH)])
                            for c in range(2):
                                for m in range(4):
                                    mo = half * 4 + m
                                    bsl = br_t[0:1, which * D + mo * P: which * D + (mo + 1) * P]
                                    ps0 = pps.tile([P, NCH], F32, tag="a")
                                    nc.tensor.matmul(ps0[:], bsl, ones_row[:], start=True, stop=False)
                                    for k in range(DS):
                                        nc.tensor.matmul(ps0[:], wsl[:, k, bass.ts(m, P)],
                                                         hT[:, k, bass.ts(c, NCH)],
                                                         start=False, stop=(k == DS - 1))
                                    nc.scalar.activation(qkT[:, qoff + mo, bass.ts(c, NCH)], ps0[:], AF.Copy)

                    # ---- V projection: out[t, o] into Vg ----
                    for half in range(2):
                        wsl = pw.tile([P, DS, NCH], BF16, tag="w")
                        nc.sync.dma_start(
                            wsl[:],
                            WvT_d[l].rearrange("(po pi) o -> pi po o", pi=P)[:, :, bass.ts(half, NCH)])
                        bvs = pbv.tile([1, NCH], BF16, tag="bvs")
                        nc.sync.dma_start(bvs[:], bv_d[:, l * D + half * NCH:l * D + (half + 1) * NCH])
                        for t_ in range(TS):
                            ps_t = pps.tile([P, NCH], F32, tag="a")
                            nc.tensor.matmul(ps_t[:], ones_row[0:1, 0:P], bvs[:],
                                             start=True, stop=False)
                            for k in range(DS):
                                nc.tensor.matmul(ps_t[:], hT[:, k, bass.ts(t_, P)],
                                                 wsl[:, k, :],
                                                 start=False, stop=(k == DS - 1))
                            nc.scalar.activation(
                                Vg[:, t_, 8 * half:8 * half + 8, 0:HD],
                                ps_t[:].rearrange("p (h d) -> p h d", d=HD), AF.Copy)

                    # ---- attention: scores(u) pipelined over PV/norm(u-1) ----
                    def attn_scores(h, c):
                        pbase = (h % 2) * HD
                        sub = h // 2
                        PT = ppt.tile([P, TS, NCH], BF16, tag="pt")
                        ntk = 4 * c + 4
                        for tk in range(ntk):
                            ls = max(0, tk * P - c * NCH)
                            w_ = NCH - ls
                            sT = pps.tile([P, NCH], F32, tag="a")
                            nc.tensor.matmul(
                                sT[:, :w_],
                                qkT[pbase:pbase + HD, DS + sub, bass.ts(tk, P)],
                                qkT[pbase:pbase + HD, sub, c * NCH + ls:(c + 1) * NCH],
                                start=True, stop=True)
                            nc.scalar.activation(PT[:, tk, ls:], sT[:, :w_], AF.Exp,
                                                 scale=float(SCALE))
                            if tk >= 4 * c:
                                nc.vector.tensor_mul(PT[:, tk, ls:ls + P],
                                                     PT[:, tk, ls:ls + P], mask[:])
                        return PT

                    def attn_pv(h, c, PT):
                        pbase = (h % 2) * HD
                        sub = h // 2
                        tch = bass.ts(c, NCH)
                        ntk = 4 * c + 4
                        po = ppo.tile([HD + 1, NCH], F32, tag="o")
                        for tk in range(ntk):
                            ls = max(0, tk * P - c * NCH)
                            nc.tensor.matmul(po[:, ls:], Vg[:, tk, h, :],
                                             PT[:, tk, ls:],
                                             start=(tk == 0), stop=(tk == ntk - 1))
                        dn = pr.tile([1, NCH], F32, tag="dn")
                        nc.vector.reciprocal(dn[:], po[HD:HD + 1, :])
                        bc = pbc.tile([HD, NCH], F32, tag="bc")
                        nc.gpsimd.partition_broadcast(bc[:], dn[:], channels=HD)
                        nc.vector.tensor_mul(yT[pbase:pbase + HD, sub, tch],
                                             po[0:HD, :], bc[:])

                    prev = None
                    for h in range(H):
                        for c in range(2):
                            PT = attn_scores(h, c)
                            if prev is not None:
                                attn_pv(*prev)
                            prev = (h, c, PT)
                    attn_pv(*prev)

                    # ---- attention out projection + residual + LN2 stats ----
                    wo0 = pw.tile([P, DS, NCH], BF16, tag="w")
                    nc.sync.dma_start(wo0[:], WoT_d[l].rearrange("(po pi) o -> pi po o", pi=P)[:, :, bass.ts(0, NCH)])
                    wo1 = pw.tile([P, DS, NCH], BF16, tag="w")
                    nc.sync.dma_start(wo1[:], WoT_d[l].rearrange("(po pi) o -> pi po o", pi=P)[:, :, bass.ts(1, NCH)])
                    stat2a = pst.tile([65, NCH], F32, tag="st")
                    stat2b = pst.tile([65, NCH], F32, tag="st")
                    stat2 = [stat2a, stat2b]
                    for c in range(2):
                        tch = bass.ts(c, NCH)
                        for mo in range(DS):
                            wsl_o = wo0 if mo < 4 else wo1
                            m = mo % 4
                            ps0 = pps.tile([P, NCH], F32, tag="a")
                            for k in range(DS):
                                nc.tensor.matmul(ps0[:], wsl_o[:, k, bass.ts(m, P)],
                                                 yT[:, k, tch],
                                                 start=(k == 0), stop=(k == DS - 1))
                            nc.vector.scalar_tensor_tensor(
                                xT[:, mo, tch], ps0[:],
                                sm[:, base + SM_BO + mo:base + SM_BO + mo + 1],
                                xT[:, mo, tch], op0=ALU.add, op1=ALU.add)
                            ln_stats_k(stat2[c], c, mo)
                        A2c, B2c = ln_finish(stat2[c])
                        ln_apply(A2c, B2c, c)


                    # ---- MLP W1 -> gelu -> gT (qkT reused) ----
                    for quarter in range(4):
                        wsl = pw.tile([P, DS, NCH], BF16, tag="w")
                        nc.sync.dma_start(
                            wsl[:],
                            W1T_d[l].rearrange("(po pi) o -> pi po o", pi=P)[:, :, bass.ts(quarter, NCH)])
                        for c in range(2):
                            for m in range(4):
                                mo = quarter * 4 + m
                                ps0 = pps.tile([P, NCH], F32, tag="a")
                                for k in range(DS):
                                    nc.tensor.matmul(ps0[:], wsl[:, k, bass.ts(m, P)],
                                                     hT[:, k, bass.ts(c, NCH)],
                                                     start=(k == 0), stop=(k == DS - 1))
                                nc.scalar.activation(
                                    qkT[:, mo, bass.ts(c, NCH)], ps0[:], AF.Gelu,
                                    bias=sm[:, base + SM_B1 + mo:base + SM_B1 + mo + 1])

                    # ---- W2 + residual + next-LN stats ----
                    stat3a = pst.tile([65, NCH], F32, tag="st")
                    stat3b = pst.tile([65, NCH], F32, tag="st")
                    stat3 = [stat3a, stat3b]
                    for c in range(2):
                        tch = bass.ts(c, NCH)
                        for quarter in range(4):
                            w2q = pw.tile([P, D2S, P * 2], BF16, tag="w")
                            nc.sync.dma_start(
                                w2q[:],
                                W2T_d[l].rearrange("(po pi) o -> pi po o", pi=P)[:, :, bass.ts(quarter, P * 2)])
                            for m in range(2):
                                mo = quarter * 2 + m
                                ps0 = pps.tile([P, NCH], F32, tag="a")
                                for k in range(D2S):
                                    nc.tensor.matmul(ps0[:], w2q[:, k, bass.ts(m, P)],
                                                     qkT[:, k, tch],
                                                     start=(k == 0), stop=(k == D2S - 1))
                                nc.vector.scalar_tensor_tensor(
                                    xT[:, mo, tch], ps0[:],
                                    sm[:, base + SM_B2 + mo:base + SM_B2 + mo + 1],
                                    xT[:, mo, tch], op0=ALU.add, op1=ALU.add)
                                ln_stats_k(stat3[c], c, mo)
                        A3c, B3c = ln_finish(stat3[c])
                        ln_apply(A3c, B3c, c)

                # ---- vocab head (final LN already applied in last W2 pass) ----
                hw_r = hT_w_d[:, :].rearrange("(po pi) v -> pi po v", pi=P)
                for vp in range(VS // 2):
                    ws0 = pw.tile([P, DS, NCH], BF16, tag="w")
                    nc.sync.dma_start(ws0[:], hw_r[:, :, bass.ts(2 * vp, NCH)])
                    ws1 = pw.tile([P, DS, NCH], BF16, tag="w")
                    nc.sync.dma_start(ws1[:], hw_r[:, :, bass.ts(2 * vp + 1, NCH)])
                    hb2 = phb.tile([1, 2 * NCH], BF16, tag="hb")
                    nc.sync.dma_start(hb2[:], hb_d[:, 2 * vp * NCH:(2 * vp + 2) * NCH])
                    for t_ in range(TS):
                        ps0 = pps.tile([P, NCH], F32, tag="a")
                        ps1 = pps.tile([P, NCH], F32, tag="a")
                        nc.tensor.matmul(ps0[:], ones_row[0:1, 0:P], hb2[0:1, 0:NCH],
                                         start=True, stop=False)
                        nc.tensor.matmul(ps1[:], ones_row[0:1, 0:P], hb2[0:1, NCH:2 * NCH],
                                         start=True, stop=False)
                        for k in range(DS):
                            nc.tensor.matmul(ps0[:], hT[:, k, bass.ts(t_, P)],
                                             ws0[:, k, :],
                                             start=False, stop=(k == DS - 1))
                            nc.tensor.matmul(ps1[:], hT[:, k, bass.ts(t_, P)],
                                             ws1[:, k, :],
                                             start=False, stop=(k == DS - 1))
                        for j, psx in ((0, ps0), (1, ps1)):
                            ot = px.tile([P, NCH], F32, tag="ot")
                            nc.scalar.activation(ot[:], psx[:], AF.Copy)
                            nc.sync.dma_start(out_r[:, t_, bass.ts(2 * vp + j, NCH)], ot[:])

    nc.compile()
    return nc


_NC = {}


def _get_nc(repeat=1):
    if repeat not in _NC:
        _NC[repeat] = _build(repeat)
    return _NC[repeat]


def _pack_cols(vec, ncols):
    """[ncols*128] -> [128, ncols] with column j = vec[j*128:(j+1)*128]."""
    return np.ascontiguousarray(vec.reshape(ncols, P).T)


def kernel(idx, timesteps, tok_emb_w, pos_emb, global_pos_emb,
           ln1_w, ln1_b, Wq, bq, Wk, bk, Wv, bv, Wo, bo,
           ln2_w, ln2_b, W1, b1, W2, b2, lnf_w, lnf_b, head_w):
    global LAST_RESULTS
    f = lambda a: np.asarray(a, dtype=np.float32)
    idx = np.asarray(idx, dtype=np.int64)
    tsteps = np.asarray(timesteps, dtype=np.int64)
    tok_emb_w, pos_emb, global_pos_emb = f(tok_emb_w), f(pos_emb), f(global_pos_emb)

    # embedding on host (tiny compute, avoids on-device gather)
    x0 = tok_emb_w[idx] + global_pos_emb[0][tsteps[:, 0]][:, None, :] + pos_emb[:, :T]
    x0 = np.ascontiguousarray(x0.astype(np.float32))

    # fold LN affine params into adjacent weights (exact rewrite)
    Wq, bq, Wk, bk, Wv, bv = f(Wq), f(bq), f(Wk), f(bk), f(Wv), f(bv)
    Wo, bo, W1, b1, W2, b2 = f(Wo), f(bo), f(W1), f(b1), f(W2), f(b2)
    ln1_w, ln1_b, ln2_w, ln2_b = f(ln1_w), f(ln1_b), f(ln2_w), f(ln2_b)
    lnf_w, lnf_b, head_w = f(lnf_w), f(lnf_b), f(head_w)

    Wqf = Wq * ln1_w[:, None, :]
    bqf = bq + np.einsum('lod,ld->lo', Wq, ln1_b)
    Wkf = Wk * ln1_w[:, None, :]
    bkf = bk + np.einsum('lod,ld->lo', Wk, ln1_b)
    Wvf = Wv * ln1_w[:, None, :]
    bvf = bv + np.einsum('lod,ld->lo', Wv, ln1_b)
    W1f = W1 * ln2_w[:, None, :]
    b1f = b1 + np.einsum('lod,ld->lo', W1, ln2_b)
    headf = head_w * lnf_w[None, :]
    hb = head_w @ lnf_b

    bf = lambda a: np.ascontiguousarray(np.asarray(a, np.float32)).astype(ml_dtypes.bfloat16)
    shared = {
        "WqT": bf(Wqf.transpose(0, 2, 1)),
        "WkT": bf(Wkf.transpose(0, 2, 1)),
        "WvT": bf(Wvf.transpose(0, 2, 1)),
        "WoT": bf(Wo.transpose(0, 2, 1)),
        "W1T": bf(W1f.transpose(0, 2, 1)),
        "W2T": bf(W2.transpose(0, 2, 1)),
        "headT": bf(headf.T),
        "bvB": bf(bvf.reshape(1, L * D)),
        "hbB": bf(hb.reshape(1, V)),
        "brows": bf(np.concatenate([bqf, bkf], axis=1)),  # [L, 2D]
    }
    smalls = np.zeros((P, SM_COLS), np.float32)
    for l in range(L):
        b_ = l * SM_PER_LAYER
        smalls[:, b_ + SM_BO:b_ + SM_BO + 8] = _pack_cols(bo[l], DS)
        smalls[:, b_ + SM_B2:b_ + SM_B2 + 8] = _pack_cols(b2[l], DS)
        smalls[:, b_ + SM_B1:b_ + SM_B1 + 16] = _pack_cols(b1f[l], D2S)
    shared["smalls"] = smalls

    m01 = (np.arange(P)[:, None] <= np.arange(P)[None, :])
    shared["mask01"] = m01.astype(ml_dtypes.bfloat16)

    in_maps = []
    for b_ in range(B):
        m = dict(shared)
        m["x0T"] = np.ascontiguousarray(x0[b_].T)
        in_maps.append(m)

    global LAST_IN_MAPS
    LAST_IN_MAPS = in_maps
    nc = _get_nc()
    res = run_bass_kernel_spmd(nc, in_maps, core_ids=list(range(B)), trace=TRACE)
    LAST_RESULTS = res
    out = np.stack([np.asarray(res.results[c]["logits"], np.float32) for c in range(B)])
    return out


# ---------------------------------------------------------------------------
# Timing helpers (test-only): replicate run_bass_via_pjrt's sharded jit with
# device-resident inputs so repeated calls measure (dispatch + NEFF exec).
# ---------------------------------------------------------------------------
LAST_IN_MAPS = None


def _sharded_exec(nc, in_maps):
    import jax
    from jax.experimental.shard_map import shard_map
    from jax.sharding import Mesh, PartitionSpec
    from concourse import bass2jax

    bass2jax.install_neuronx_cc_hook()
    n_cores = len(in_maps)
    partition_name = nc.partition_id_tensor.name if nc.partition_id_tensor else None
    in_names, out_names, out_avals, zero_outs = [], [], [], []
    for alloc in nc.m.functions[0].allocations:
        if not isinstance(alloc, mybir.MemoryLocationSet):
            continue
        name = alloc.memorylocations[0].name
        if alloc.kind == "ExternalInput":
            if name != partition_name:
                in_names.append(name)
        elif alloc.kind == "ExternalOutput":
            shape = tuple(alloc.tensor_shape)
            dtype = mybir.dt.np(alloc.dtype)
            out_names.append(name)
            out_avals.append(jax.core.ShapedArray(shape, dtype))
            zero_outs.append(np.zeros(shape, dtype))
    n_params = len(in_names)
    n_outs = len(out_avals)
    all_in_names = list(in_names) + list(out_names)
    if partition_name is not None:
        all_in_names.append(partition_name)
    donate = tuple(range(n_params, n_params + n_outs))

    def _body(*args):
        operands = list(args)
        if partition_name is not None:
            operands.append(bass2jax.partition_id_tensor())
        outs = bass2jax._bass_exec_p.bind(
            *operands,
            out_avals=tuple(out_avals),
            in_names=tuple(all_in_names),
            out_names=tuple(out_names),
            lowering_input_output_aliases=(),
            sim_require_finite=True,
            sim_require_nnan=True,
            nc=nc,
        )
        return tuple(outs)

    devices = jax.devices()[:n_cores]
    mesh = Mesh(np.asarray(devices), ("core",))
    sharded = jax.jit(
        shard_map(_body, mesh=mesh,
                  in_specs=(PartitionSpec("core"),) * (n_params + n_outs),
                  out_specs=(PartitionSpec("core"),) * n_outs,
                  check_rep=False),
        donate_argnums=donate, keep_unused=True)

    concat_in = [np.concatenate([np.asarray(m[name]) for m in in_maps], axis=0)
                 for name in in_names]
    concat_zeros = [np.zeros((n_cores * z.shape[0], *z.shape[1:]), z.dtype)
                    for z in zero_outs]
    from jax.sharding import NamedSharding
    sh = NamedSharding(mesh, PartitionSpec("core"))
    dev_in = [jax.device_put(a, sh) for a in concat_in]
    return sharded, dev_in, concat_zeros, sh


def _time_exec(nc, in_maps, iters):
    import time as _time
    import jax
    sharded, dev_in, concat_zeros, sh = _sharded_exec(nc, in_maps)
    times = []
    for _ in range(iters):
        zs = [jax.device_put(z, sh) for z in concat_zeros]
        jax.block_until_ready(zs)
        jax.block_until_ready(dev_in)
        t0 = _time.perf_counter()
        out = sharded(*dev_in, *zs)
        jax.block_until_ready(out)
        times.append(_time.perf_counter() - t0)
    return times


def timed_run(iters=5):
    assert LAST_IN_MAPS is not None, "call kernel() first"
    return _time_exec(_get_nc(), LAST_IN_MAPS, iters)


def timed_slope(ns=(1, 4, 12), zsets=12):
    """Async-dispatch n calls back-to-back; slope of total-time vs n ~ exec."""
    import time as _time
    import jax
    assert LAST_IN_MAPS is not None
    sharded, dev_in, concat_zeros, sh = _sharded_exec(_get_nc(), LAST_IN_MAPS)
    all_zs = [[jax.device_put(z, sh) for z in concat_zeros] for _ in range(zsets)]
    jax.block_until_ready(all_zs)
    jax.block_until_ready(dev_in)
    # warm
    out = sharded(*dev_in, *all_zs[0])
    jax.block_until_ready(out)
    res = {}
    for n in ns:
        zs_fresh = [[jax.device_put(z, sh) for z in concat_zeros] for _ in range(n)]
        jax.block_until_ready(zs_fresh)
        t0 = _time.perf_counter()
        outs = [sharded(*dev_in, *zs_fresh[i]) for i in range(n)]
        jax.block_until_ready(outs)
        res[n] = _time.perf_counter() - t0
    return res


def timed_repeat(r=5, iters=6):
    """exec_ns ~= (min_time(R=r NEFF) - min_time(R=1 NEFF)) / (r-1)."""
    assert LAST_IN_MAPS is not None
    t1 = min(_time_exec(_get_nc(1), LAST_IN_MAPS, iters))
    tr = min(_time_exec(_get_nc(r), LAST_IN_MAPS, iters))
    return (tr - t1) / (r - 1), t1, tr


# revision 14
# speedup vs baseline: 1.0639x; 1.0108x over previous
"""GPT forward kernel for 8 TRN2 NeuronCores (v2).

Data-parallel over batch (B=8 -> 1 sequence per core). Host folds the LN
affine params into the adjacent weights (exact), pre-transposes weights to
put the contraction dim on SBUF partitions, casts them to bf16, and does
the embedding gather. On device the residual stream is kept transposed
(x^T [D, T] fp32 in SBUF); LayerNorm is pure normalization whose stats
matmuls are interleaved into the producing GEMM, broadcast planes are
built on GpSimd, and PSUM drains ride the Scalar engine so the Tensor
engine stays dense.
"""
import sys
sys.path.insert(0, '/opt/trn_rl_repo')
import numpy as np
import ml_dtypes

import concourse.bass as bass
import concourse.tile as tile
from concourse import bacc, mybir
from concourse.bass_utils import run_bass_kernel_spmd

B, T, D, H, L, V, MAXT = 8, 1024, 1024, 16, 8, 8192, 4096
HD = D // H          # 64
P = 128
DS = D // P          # 8 d-subtiles
TS = T // P          # 8 t-subtiles
D2S = (2 * D) // P   # 16 mlp subtiles
VS = V // 512        # 16 vocab chunks
NCH = 512
EPS = 1e-5
SCALE = 1.0 / np.sqrt(HD)

F32 = mybir.dt.float32
BF16 = mybir.dt.bfloat16
AF = mybir.ActivationFunctionType
ALU = mybir.AluOpType

# smalls[:, col] layout, per layer base = l*32  (bo, b2: [P,1]-packed cols)
SM_BO, SM_B2, SM_B1 = 0, 8, 16
SM_PER_LAYER = 32
SM_COLS = L * SM_PER_LAYER

TRACE = False
LAST_RESULTS = None


def _build(repeat=1):
    import contextlib
    nc = bacc.Bacc("TRN2", target_bir_lowering=False)

    x0T_d = nc.dram_tensor("x0T", [D, T], F32, kind="ExternalInput")
    WqT_d = nc.dram_tensor("WqT", [L, D, D], BF16, kind="ExternalInput")
    WkT_d = nc.dram_tensor("WkT", [L, D, D], BF16, kind="ExternalInput")
    WvT_d = nc.dram_tensor("WvT", [L, D, D], BF16, kind="ExternalInput")
    WoT_d = nc.dram_tensor("WoT", [L, D, D], BF16, kind="ExternalInput")
    W1T_d = nc.dram_tensor("W1T", [L, D, 2 * D], BF16, kind="ExternalInput")
    W2T_d = nc.dram_tensor("W2T", [L, 2 * D, D], BF16, kind="ExternalInput")
    hT_w_d = nc.dram_tensor("headT", [D, V], BF16, kind="ExternalInput")
    sm_d = nc.dram_tensor("smalls", [P, SM_COLS], F32, kind="ExternalInput")
    brow_d = nc.dram_tensor("brows", [L, 2 * D], BF16, kind="ExternalInput")
    bv_d = nc.dram_tensor("bvB", [1, L * D], BF16, kind="ExternalInput")
    hb_d = nc.dram_tensor("hbB", [1, V], BF16, kind="ExternalInput")
    mask_d = nc.dram_tensor("mask01", [P, P], BF16, kind="ExternalInput")
    out_d = nc.dram_tensor("logits", [T, V], F32, kind="ExternalOutput")

    out_r = out_d[:, :].rearrange("(t pi) v -> pi t v", pi=P)

    with tile.TileContext(nc) as tc:
        with (
            tc.tile_pool(name="pc", bufs=1) as pc,
            tc.tile_pool(name="pw", bufs=3) as pw,
            tc.tile_pool(name="pbv", bufs=2) as pbv,
            tc.tile_pool(name="pbr", bufs=2) as pbr,
            tc.tile_pool(name="phb", bufs=2) as phb,
            tc.tile_pool(name="ppt", bufs=2) as ppt,
            tc.tile_pool(name="px", bufs=3) as px,
            tc.tile_pool(name="pr", bufs=3) as pr,
            tc.tile_pool(name="pln", bufs=4) as pln,
            tc.tile_pool(name="pbc", bufs=2) as pbc,
            tc.tile_pool(name="pps", bufs=4, space="PSUM") as pps,
            tc.tile_pool(name="ppo", bufs=2, space="PSUM") as ppo,
            tc.tile_pool(name="pst", bufs=2, space="PSUM") as pst,
        ):
            xT = pc.tile([P, DS, T], F32)     # residual, transposed
            xb = pc.tile([P, DS, T], BF16)    # bf16 copy of residual (LN input)
            hT = pc.tile([P, DS, T], BF16)    # LN output; reused as attn y^T
            qkT = pc.tile([P, 2 * DS, T], BF16)  # q rows 0:8, k rows 8:16; reused as gT
            Vg = pc.tile([P, TS, H, HD + 1], BF16)
            yT = hT
            sm = pc.tile([P, SM_COLS], F32)
            mask = pc.tile([P, P], BF16)
            ones_row = pc.tile([1, NCH], BF16)
            ones_cb = pc.tile([P, 1], BF16)
            eps_t = pc.tile([1, 1], F32)

            nc.vector.memset(ones_row[:], 1.0)
            nc.vector.memset(ones_cb[:], 1.0)
            nc.vector.memset(eps_t[:], EPS)
            nc.vector.memset(Vg[:, :, :, HD:HD + 1], 1.0)
            nc.sync.dma_start(sm[:], sm_d[:, :])
            nc.sync.dma_start(mask[:], mask_d[:, :])

            def ln_stats_k(stat2c, c, k):
                """stat2c: PSUM [65, NCH]; row 0 accumulates sum, row 64 sumsq."""
                tch = bass.ts(c, NCH)
                nc.vector.tensor_copy(xb[:, k, tch], xT[:, k, tch])
                sq = px.tile([P, NCH], BF16, tag="sq")
                nc.scalar.activation(sq[:], xb[:, k, tch], AF.Square)
                nc.tensor.matmul(stat2c[0:1, :], ones_cb[:],
                                 xb[:, k, tch], start=(k == 0),
                                 stop=(k == DS - 1), skip_group_check=True)
                nc.tensor.matmul(stat2c[64:65, :], ones_cb[:],
                                 sq[:], start=(k == 0), stop=(k == DS - 1),
                                 skip_group_check=True)

            def ln_finish(stat2c):
                """-> (A, B) bf16 [P, NCH] planes: h = x*A + B."""
                m2 = pr.tile([1, NCH], F32, tag="r")
                nc.scalar.activation(m2[:], stat2c[0:1, :],
                                     AF.Square, scale=1.0 / D)
                var = pr.tile([1, NCH], F32, tag="r")
                nc.vector.scalar_tensor_tensor(var[:], stat2c[64:65, :],
                                               1.0 / D, m2[:],
                                               op0=ALU.mult, op1=ALU.subtract)
                sd = pr.tile([1, NCH], F32, tag="r")
                nc.scalar.activation(sd[:], var[:], AF.Sqrt, bias=eps_t[:])
                srow = pr.tile([1, NCH], BF16, tag="rb")
                nm = pr.tile([1, NCH], BF16, tag="rb")
                with nc.allow_low_precision(reason="LN planes applied in bf16 anyway"):
                    nc.vector.reciprocal(srow[:], sd[:])
                    nc.vector.scalar_tensor_tensor(nm[:], stat2c[0:1, :],
                                                   -1.0 / D, srow[:],
                                                   op0=ALU.mult, op1=ALU.mult)
                A = pln.tile([P, NCH], BF16, tag="pl")
                nc.gpsimd.partition_broadcast(A[:], srow[:], channels=P)
                Bp = pln.tile([P, NCH], BF16, tag="pl")
                nc.gpsimd.partition_broadcast(Bp[:], nm[:], channels=P)
                return A, Bp

            def ln_apply(A, Bp, c):
                tch = bass.ts(c, NCH)
                for k in range(DS):
                    tmp = px.tile([P, NCH], BF16, tag="lt")
                    nc.vector.tensor_mul(tmp[:], xb[:, k, tch], A[:])
                    nc.vector.tensor_add(hT[:, k, tch], tmp[:], Bp[:])

            loop_cm = tc.For_i(0, repeat, 1) if repeat > 1 else contextlib.nullcontext()
            with loop_cm:
                nc.sync.dma_start(xT[:], x0T_d[:, :].rearrange("(po pi) t -> pi po t", pi=P))
                stat0 = pst.tile([65, NCH], F32, tag="st")
                stat1 = pst.tile([65, NCH], F32, tag="st")
                stat = [stat0, stat1]
                for c in range(2):
                    for k in range(DS):
                        ln_stats_k(stat[c], c, k)
                    A0c, B0c = ln_finish(stat[c])
                    ln_apply(A0c, B0c, c)

                for l in range(L):
                    base = l * SM_PER_LAYER
                    br_t = pbr.tile([1, 2 * D], BF16, tag="br")
                    nc.sync.dma_start(br_t[:], brow_d[l:l + 1, :])

                    # ---- q^T / k^T projections (bias via K=1 matmul row) ----
                    for which, W_d in ((0, WqT_d), (1, WkT_d)):
                        qoff = which * DS
                        for half in range(2):
                            wsl = pw.tile([P, DS, NCH], BF16, tag="w")
                            nc.sync.dma_start(
                                wsl[:],
                                W_d[l].rearrange("(po pi) o -> pi po o", pi=P)[:, :, bass.ts(half, NCH)])
                            for c in range(2):
                                for m in range(4):
                                    mo = half * 4 + m
                                    bsl = br_t[0:1, which * D + mo * P: which * D + (mo + 1) * P]
                                    ps0 = pps.tile([P, NCH], F32, tag="a")
                                    nc.tensor.matmul(ps0[:], bsl, ones_row[:], start=True, stop=False)
                                    for k in range(DS):
                                        nc.tensor.matmul(ps0[:], wsl[:, k, bass.ts(m, P)],
                                                         hT[:, k, bass.ts(c, NCH)],
                                                         start=False, stop=(k == DS - 1))
                                    nc.scalar.activation(qkT[:, qoff + mo, bass.ts(c, NCH)], ps0[:], AF.Copy)

                    # ---- V projection: out[t, o] into Vg ----
                    for half in range(2):
                        wsl = pw.tile([P, DS, NCH], BF16, tag="w")
                        nc.sync.dma_start(
                            wsl[:],
                            WvT_d[l].rearrange("(po pi) o -> pi po o", pi=P)[:, :, bass.ts(half, NCH)])
                        bvs = pbv.tile([1, NCH], BF16, tag="bvs")
                        nc.sync.dma_start(bvs[:], bv_d[:, l * D + half * NCH:l * D + (half + 1) * NCH])
                        for t_ in range(TS):
                            ps_t = pps.tile([P, NCH], F32, tag="a")
                            nc.tensor.matmul(ps_t[:], ones_row[0:1, 0:P], bvs[:],
                                             start=True, stop=False)
                            for k in range(DS):
                                nc.tensor.matmul(ps_t[:], hT[:, k, bass.ts(t_, P)],
                                                 wsl[:, k, :],
                                                 start=False, stop=(k == DS - 1))
                            nc.scalar.activation(
                                Vg[:, t_, 8 * half:8 * half + 8, 0:HD],
                                ps_t[:].rearrange("p (h d) -> p h d", d=HD), AF.Copy)

                    # ---- attention: scores(u) pipelined over PV/norm(u-1) ----
                    def attn_scores(h, c):
                        pbase = (h % 2) * HD
                        sub = h // 2
                        PT = ppt.tile([P, TS, NCH], BF16, tag="pt")
                        ntk = 4 * c + 4
                        for tk in range(ntk):
                            ls = max(0, tk * P - c * NCH)
                            w_ = NCH - ls
                            sT = pps.tile([P, NCH], F32, tag="a")
                            nc.tensor.matmul(
                                sT[:, :w_],
                                qkT[pbase:pbase + HD, DS + sub, bass.ts(tk, P)],
                                qkT[pbase:pbase + HD, sub, c * NCH + ls:(c + 1) * NCH],
                                start=True, stop=True)
                            nc.scalar.activation(PT[:, tk, ls:], sT[:, :w_], AF.Exp,
                                                 scale=float(SCALE))
                            if tk >= 4 * c:
                                nc.vector.tensor_mul(PT[:, tk, ls:ls + P],
                                                     PT[:, tk, ls:ls + P], mask[:])
                        return PT

                    def attn_pv(h, c, PT):
                        pbase = (h % 2) * HD
                        sub = h // 2
                        tch = bass.ts(c, NCH)
                        ntk = 4 * c + 4
                        po = ppo.tile([HD + 1, NCH], F32, tag="o")
                        for tk in range(ntk):
                            ls = max(0, tk * P - c * NCH)
                            nc.tensor.matmul(po[:, ls:], Vg[:, tk, h, :],
                                             PT[:, tk, ls:],
                                             start=(tk == 0), stop=(tk == ntk - 1))
                        dn = pr.tile([1, NCH], F32, tag="dn")
                        nc.vector.reciprocal(dn[:], po[HD:HD + 1, :])
                        bc = pbc.tile([HD, NCH], F32, tag="bc")
                        nc.gpsimd.partition_broadcast(bc[:], dn[:], channels=HD)
                        nc.vector.tensor_mul(yT[pbase:pbase + HD, sub, tch],
                                             po[0:HD, :], bc[:])

                    # o-proj weights + LN2 stat tiles (o-proj c0 groups are
                    # interleaved into the attention c1 units to keep PE fed
                    # while ACT works through the exp stream)
                    wo0 = pw.tile([P, DS, NCH], BF16, tag="w")
                    nc.sync.dma_start(wo0[:], WoT_d[l].rearrange("(po pi) o -> pi po o", pi=P)[:, :, bass.ts(0, NCH)])
                    wo1 = pw.tile([P, DS, NCH], BF16, tag="w")
                    nc.sync.dma_start(wo1[:], WoT_d[l].rearrange("(po pi) o -> pi po o", pi=P)[:, :, bass.ts(1, NCH)])
                    stat2a = pst.tile([65, NCH], F32, tag="st")
                    stat2b = pst.tile([65, NCH], F32, tag="st")
                    stat2 = [stat2a, stat2b]

                    def oproj_group(mo, c):
                        tch = bass.ts(c, NCH)
                        wsl_o = wo0 if mo < 4 else wo1
                        m = mo % 4
                        ps0 = pps.tile([P, NCH], F32, tag="a")
                        for k in range(DS):
                            nc.tensor.matmul(ps0[:], wsl_o[:, k, bass.ts(m, P)],
                                             yT[:, k, tch],
                                             start=(k == 0), stop=(k == DS - 1))
                        nc.vector.scalar_tensor_tensor(
                            xT[:, mo, tch], ps0[:],
                            sm[:, base + SM_BO + mo:base + SM_BO + mo + 1],
                            xT[:, mo, tch], op0=ALU.add, op1=ALU.add)
                        ln_stats_k(stat2[c], c, mo)

                    prev = None
                    for h in range(H):
                        PT = attn_scores(h, 0)
                        if prev is not None:
                            attn_pv(*prev)
                        prev = (h, 0, PT)
                    for h in range(H):
                        PT = attn_scores(h, 1)
                        attn_pv(*prev)
                        prev = (h, 1, PT)
                        if h % 2 == 1:
                            oproj_group(h // 2, 0)
                    attn_pv(*prev)
                    A2c, B2c = ln_finish(stat2[0])
                    ln_apply(A2c, B2c, 0)
                    for mo in range(DS):
                        oproj_group(mo, 1)
                    A2c, B2c = ln_finish(stat2[1])
                    ln_apply(A2c, B2c, 1)


                    # ---- MLP W1 -> gelu -> gT (qkT reused) ----
                    for quarter in range(4):
                        wsl = pw.tile([P, DS, NCH], BF16, tag="w")
                        nc.sync.dma_start(
                            wsl[:],
                            W1T_d[l].rearrange("(po pi) o -> pi po o", pi=P)[:, :, bass.ts(quarter, NCH)])
                        for c in range(2):
                            for m in range(4):
                                mo = quarter * 4 + m
                                ps0 = pps.tile([P, NCH], F32, tag="a")
                                for k in range(DS):
                                    nc.tensor.matmul(ps0[:], wsl[:, k, bass.ts(m, P)],
                                                     hT[:, k, bass.ts(c, NCH)],
                                                     start=(k == 0), stop=(k == DS - 1))
                                nc.scalar.activation(
                                    qkT[:, mo, bass.ts(c, NCH)], ps0[:], AF.Gelu,
                                    bias=sm[:, base + SM_B1 + mo:base + SM_B1 + mo + 1])

                    # ---- W2 + residual + next-LN stats ----
                    stat3a = pst.tile([65, NCH], F32, tag="st")
                    stat3b = pst.tile([65, NCH], F32, tag="st")
                    stat3 = [stat3a, stat3b]
                    for c in range(2):
                        tch = bass.ts(c, NCH)
                        for quarter in range(4):
                            w2q = pw.tile([P, D2S, P * 2], BF16, tag="w")
                            nc.sync.dma_start(
                                w2q[:],
                                W2T_d[l].rearrange("(po pi) o -> pi po o", pi=P)[:, :, bass.ts(quarter, P * 2)])
                            for m in range(2):
                                mo = quarter * 2 + m
                                ps0 = pps.tile([P, NCH], F32, tag="a")
                                for k in range(D2S):
                                    nc.tensor.matmul(ps0[:], w2q[:, k, bass.ts(m, P)],
                                                     qkT[:, k, tch],
                                                     start=(k == 0), stop=(k == D2S - 1))
                                nc.vector.scalar_tensor_tensor(
                                    xT[:, mo, tch], ps0[:],
                                    sm[:, base + SM_B2 + mo:base + SM_B2 + mo + 1],
                                    xT[:, mo, tch], op0=ALU.add, op1=ALU.add)
                                ln_stats_k(stat3[c], c, mo)
                        A3c, B3c = ln_finish(stat3[c])
                        ln_apply(A3c, B3c, c)

                # ---- vocab head (final LN already applied in last W2 pass) ----
                hw_r = hT_w_d[:, :].rearrange("(po pi) v -> pi po v", pi=P)
                for vp in range(VS // 2):
                    ws0 = pw.tile([P, DS, NCH], BF16, tag="w")
                    nc.sync.dma_start(ws0[:], hw_r[:, :, bass.ts(2 * vp, NCH)])
                    ws1 = pw.tile([P, DS, NCH], BF16, tag="w")
                    nc.sync.dma_start(ws1[:], hw_r[:, :, bass.ts(2 * vp + 1, NCH)])
                    hb2 = phb.tile([1, 2 * NCH], BF16, tag="hb")
                    nc.sync.dma_start(hb2[:], hb_d[:, 2 * vp * NCH:(2 * vp + 2) * NCH])
                    for t_ in range(TS):
                        ps0 = pps.tile([P, NCH], F32, tag="a")
                        ps1 = pps.tile([P, NCH], F32, tag="a")
                        nc.tensor.matmul(ps0[:], ones_row[0:1, 0:P], hb2[0:1, 0:NCH],
                                         start=True, stop=False)
                        nc.tensor.matmul(ps1[:], ones_row[0:1, 0:P], hb2[0:1, NCH:2 * NCH],
                                         start=True, stop=False)
                        for k in range(DS):
                            nc.tensor.matmul(ps0[:], hT[:, k, bass.ts(t_, P)],
                                             ws0[:, k, :],
                                             start=False, stop=(k == DS - 1))
                            nc.tensor.matmul(ps1[:], hT[:, k, bass.ts(t_, P)],
                                             ws1[:, k, :],
                                             start=False, stop=(k == DS - 1))
                        for j, psx in ((0, ps0), (1, ps1)):
                            ot = px.tile([P, NCH], F32, tag="ot")
                            nc.scalar.activation(ot[:], psx[:], AF.Copy)
                            nc.sync.dma_start(out_r[:, t_, bass.ts(2 * vp + j, NCH)], ot[:])

    nc.compile()
    return nc


_NC = {}


def _get_nc(repeat=1):
    if repeat not in _NC:
        _NC[repeat] = _build(repeat)
    return _NC[repeat]


def _pack_cols(vec, ncols):
    """[ncols*128] -> [128, ncols] with column j = vec[j*128:(j+1)*128]."""
    return np.ascontiguousarray(vec.reshape(ncols, P).T)


def kernel(idx, timesteps, tok_emb_w, pos_emb, global_pos_emb,
           ln1_w, ln1_b, Wq, bq, Wk, bk, Wv, bv, Wo, bo,
           ln2_w, ln2_b, W1, b1, W2, b2, lnf_w, lnf_b, head_w):
    global LAST_RESULTS
    f = lambda a: np.asarray(a, dtype=np.float32)
    idx = np.asarray(idx, dtype=np.int64)
    tsteps = np.asarray(timesteps, dtype=np.int64)
    tok_emb_w, pos_emb, global_pos_emb = f(tok_emb_w), f(pos_emb), f(global_pos_emb)

    # embedding on host (tiny compute, avoids on-device gather)
    x0 = tok_emb_w[idx] + global_pos_emb[0][tsteps[:, 0]][:, None, :] + pos_emb[:, :T]
    x0 = np.ascontiguousarray(x0.astype(np.float32))

    # fold LN affine params into adjacent weights (exact rewrite)
    Wq, bq, Wk, bk, Wv, bv = f(Wq), f(bq), f(Wk), f(bk), f(Wv), f(bv)
    Wo, bo, W1, b1, W2, b2 = f(Wo), f(bo), f(W1), f(b1), f(W2), f(b2)
    ln1_w, ln1_b, ln2_w, ln2_b = f(ln1_w), f(ln1_b), f(ln2_w), f(ln2_b)
    lnf_w, lnf_b, head_w = f(lnf_w), f(lnf_b), f(head_w)

    Wqf = Wq * ln1_w[:, None, :]
    bqf = bq + np.einsum('lod,ld->lo', Wq, ln1_b)
    Wkf = Wk * ln1_w[:, None, :]
    bkf = bk + np.einsum('lod,ld->lo', Wk, ln1_b)
    Wvf = Wv * ln1_w[:, None, :]
    bvf = bv + np.einsum('lod,ld->lo', Wv, ln1_b)
    W1f = W1 * ln2_w[:, None, :]
    b1f = b1 + np.einsum('lod,ld->lo', W1, ln2_b)
    headf = head_w * lnf_w[None, :]
    hb = head_w @ lnf_b

    bf = lambda a: np.ascontiguousarray(np.asarray(a, np.float32)).astype(ml_dtypes.bfloat16)
    shared = {
        "WqT": bf(Wqf.transpose(0, 2, 1)),
        "WkT": bf(Wkf.transpose(0, 2, 1)),
        "WvT": bf(Wvf.transpose(0, 2, 1)),
        "WoT": bf(Wo.transpose(0, 2, 1)),
        "W1T": bf(W1f.transpose(0, 2, 1)),
        "W2T": bf(W2.transpose(0, 2, 1)),
        "headT": bf(headf.T),
        "bvB": bf(bvf.reshape(1, L * D)),
        "hbB": bf(hb.reshape(1, V)),
        "brows": bf(np.concatenate([bqf, bkf], axis=1)),  # [L, 2D]
    }
    smalls = np.zeros((P, SM_COLS), np.float32)
    for l in range(L):
        b_ = l * SM_PER_LAYER
        smalls[:, b_ + SM_BO:b_ + SM_BO + 8] = _pack_cols(bo[l], DS)
        smalls[:, b_ + SM_B2:b_ + SM_B2 + 8] = _pack_cols(b2[l], DS)
        smalls[:, b_ + SM_B1:b_ + SM_B1 + 16] = _pack_cols(b1f[l], D2S)
    shared["smalls"] = smalls

    m01 = (np.arange(P)[:, None] <= np.arange(P)[None, :])
    shared["mask01"] = m01.astype(ml_dtypes.bfloat16)

    in_maps = []
    for b_ in range(B):
        m = dict(shared)
        m["x0T"] = np.ascontiguousarray(x0[b_].T)
        in_maps.append(m)

    global LAST_IN_MAPS
    LAST_IN_MAPS = in_maps
    nc = _get_nc()
    res = run_bass_kernel_spmd(nc, in_maps, core_ids=list(range(B)), trace=TRACE)
    LAST_RESULTS = res
    out = np.stack([np.asarray(res.results[c]["logits"], np.float32) for c in range(B)])
    return out


# ---------------------------------------------------------------------------
# Timing helpers (test-only): replicate run_bass_via_pjrt's sharded jit with
# device-resident inputs so repeated calls measure (dispatch + NEFF exec).
# ---------------------------------------------------------------------------
LAST_IN_MAPS = None


def _sharded_exec(nc, in_maps):
    import jax
    from jax.experimental.shard_map import shard_map
    from jax.sharding import Mesh, PartitionSpec
    from concourse import bass2jax

    bass2jax.install_neuronx_cc_hook()
    n_cores = len(in_maps)
    partition_name = nc.partition_id_tensor.name if nc.partition_id_tensor else None
    in_names, out_names, out_avals, zero_outs = [], [], [], []
    for alloc in nc.m.functions[0].allocations:
        if not isinstance(alloc, mybir.MemoryLocationSet):
            continue
        name = alloc.memorylocations[0].name
        if alloc.kind == "ExternalInput":
            if name != partition_name:
                in_names.append(name)
        elif alloc.kind == "ExternalOutput":
            shape = tuple(alloc.tensor_shape)
            dtype = mybir.dt.np(alloc.dtype)
            out_names.append(name)
            out_avals.append(jax.core.ShapedArray(shape, dtype))
            zero_outs.append(np.zeros(shape, dtype))
    n_params = len(in_names)
    n_outs = len(out_avals)
    all_in_names = list(in_names) + list(out_names)
    if partition_name is not None:
        all_in_names.append(partition_name)
    donate = tuple(range(n_params, n_params + n_outs))

    def _body(*args):
        operands = list(args)
        if partition_name is not None:
            operands.append(bass2jax.partition_id_tensor())
        outs = bass2jax._bass_exec_p.bind(
            *operands,
            out_avals=tuple(out_avals),
            in_names=tuple(all_in_names),
            out_names=tuple(out_names),
            lowering_input_output_aliases=(),
            sim_require_finite=True,
            sim_require_nnan=True,
            nc=nc,
        )
        return tuple(outs)

    devices = jax.devices()[:n_cores]
    mesh = Mesh(np.asarray(devices), ("core",))
    sharded = jax.jit(
        shard_map(_body, mesh=mesh,
                  in_specs=(PartitionSpec("core"),) * (n_params + n_outs),
                  out_specs=(PartitionSpec("core"),) * n_outs,
                  check_rep=False),
        donate_argnums=donate, keep_unused=True)

    concat_in = [np.concatenate([np.asarray(m[name]) for m in in_maps], axis=0)
                 for name in in_names]
    concat_zeros = [np.zeros((n_cores * z.shape[0], *z.shape[1:]), z.dtype)
                    for z in zero_outs]
    from jax.sharding import NamedSharding
    sh = NamedSharding(mesh, PartitionSpec("core"))
    dev_in = [jax.device_put(a, sh) for a in concat_in]
    return sharded, dev_in, concat_zeros, sh


def _time_exec(nc, in_maps, iters):
    import time as _time
    import jax
    sharded, dev_in, concat_zeros, sh = _sharded_exec(nc, in_maps)
    times = []
    for _ in range(iters):
        zs = [jax.device_put(z, sh) for z in concat_zeros]
        jax.block_until_ready(zs)
        jax.block_until_ready(dev_in)
        t0 = _time.perf_counter()
        out = sharded(*dev_in, *zs)
        jax.block_until_ready(out)
        times.append(_time.perf_counter() - t0)
    return times


def timed_run(iters=5):
    assert LAST_IN_MAPS is not None, "call kernel() first"
    return _time_exec(_get_nc(), LAST_IN_MAPS, iters)


def timed_slope(ns=(1, 4, 12), zsets=12):
    """Async-dispatch n calls back-to-back; slope of total-time vs n ~ exec."""
    import time as _time
    import jax
    assert LAST_IN_MAPS is not None
    sharded, dev_in, concat_zeros, sh = _sharded_exec(_get_nc(), LAST_IN_MAPS)
    all_zs = [[jax.device_put(z, sh) for z in concat_zeros] for _ in range(zsets)]
    jax.block_until_ready(all_zs)
    jax.block_until_ready(dev_in)
    # warm
    out = sharded(*dev_in, *all_zs[0])
    jax.block_until_ready(out)
    res = {}
    for n in ns:
        zs_fresh = [[jax.device_put(z, sh) for z in concat_zeros] for _ in range(n)]
        jax.block_until_ready(zs_fresh)
        t0 = _time.perf_counter()
        outs = [sharded(*dev_in, *zs_fresh[i]) for i in range(n)]
        jax.block_until_ready(outs)
        res[n] = _time.perf_counter() - t0
    return res


def timed_repeat(r=5, iters=6):
    """exec_ns ~= (min_time(R=r NEFF) - min_time(R=1 NEFF)) / (r-1)."""
    assert LAST_IN_MAPS is not None
    t1 = min(_time_exec(_get_nc(1), LAST_IN_MAPS, iters))
    tr = min(_time_exec(_get_nc(r), LAST_IN_MAPS, iters))
    return (tr - t1) / (r - 1), t1, tr


# revision 15
# speedup vs baseline: 1.1059x; 1.0395x over previous
"""GPT forward kernel for 8 TRN2 NeuronCores (v2).

Data-parallel over batch (B=8 -> 1 sequence per core). Host folds the LN
affine params into the adjacent weights (exact), pre-transposes weights to
put the contraction dim on SBUF partitions, casts them to bf16, and does
the embedding gather. On device the residual stream is kept transposed
(x^T [D, T] fp32 in SBUF); LayerNorm is pure normalization whose stats
matmuls are interleaved into the producing GEMM, broadcast planes are
built on GpSimd, and PSUM drains ride the Scalar engine so the Tensor
engine stays dense.
"""
import sys
sys.path.insert(0, '/opt/trn_rl_repo')
import numpy as np
import ml_dtypes

import concourse.bass as bass
import concourse.tile as tile
from concourse import bacc, mybir
from concourse.bass_utils import run_bass_kernel_spmd

B, T, D, H, L, V, MAXT = 8, 1024, 1024, 16, 8, 8192, 4096
HD = D // H          # 64
P = 128
DS = D // P          # 8 d-subtiles
TS = T // P          # 8 t-subtiles
D2S = (2 * D) // P   # 16 mlp subtiles
VS = V // 512        # 16 vocab chunks
NCH = 512
EPS = 1e-5
SCALE = 1.0 / np.sqrt(HD)

F32 = mybir.dt.float32
BF16 = mybir.dt.bfloat16
AF = mybir.ActivationFunctionType
ALU = mybir.AluOpType

# smalls[:, col] layout, per layer base = l*32  (bo, b2: [P,1]-packed cols)
SM_BO, SM_B2, SM_B1 = 0, 8, 16
SM_PER_LAYER = 32
SM_COLS = L * SM_PER_LAYER

TRACE = False
LAST_RESULTS = None


def _build(repeat=1, biases=True):
    import contextlib
    nc = bacc.Bacc("TRN2", target_bir_lowering=False)

    x0T_d = nc.dram_tensor("x0T", [D, T], F32, kind="ExternalInput")
    WqT_d = nc.dram_tensor("WqT", [L, D, D], BF16, kind="ExternalInput")
    WkT_d = nc.dram_tensor("WkT", [L, D, D], BF16, kind="ExternalInput")
    WvT_d = nc.dram_tensor("WvT", [L, D, D], BF16, kind="ExternalInput")
    WoT_d = nc.dram_tensor("WoT", [L, D, D], BF16, kind="ExternalInput")
    W1T_d = nc.dram_tensor("W1T", [L, D, 2 * D], BF16, kind="ExternalInput")
    W2T_d = nc.dram_tensor("W2T", [L, 2 * D, D], BF16, kind="ExternalInput")
    hT_w_d = nc.dram_tensor("headT", [D, V], BF16, kind="ExternalInput")
    sm_d = nc.dram_tensor("smalls", [P, SM_COLS], F32, kind="ExternalInput")
    brow_d = nc.dram_tensor("brows", [L, 2 * D], BF16, kind="ExternalInput")
    bv_d = nc.dram_tensor("bvB", [1, L * D], BF16, kind="ExternalInput")
    hb_d = nc.dram_tensor("hbB", [1, V], BF16, kind="ExternalInput")
    mask_d = nc.dram_tensor("mask01", [P, P], BF16, kind="ExternalInput")
    out_d = nc.dram_tensor("logits", [T, V], F32, kind="ExternalOutput")

    out_r = out_d[:, :].rearrange("(t pi) v -> pi t v", pi=P)

    with tile.TileContext(nc) as tc:
        with (
            tc.tile_pool(name="pc", bufs=1) as pc,
            tc.tile_pool(name="pw", bufs=3) as pw,
            tc.tile_pool(name="pbv", bufs=2) as pbv,
            tc.tile_pool(name="pbr", bufs=2) as pbr,
            tc.tile_pool(name="phb", bufs=2) as phb,
            tc.tile_pool(name="ppt", bufs=2) as ppt,
            tc.tile_pool(name="px", bufs=3) as px,
            tc.tile_pool(name="pr", bufs=3) as pr,
            tc.tile_pool(name="pln", bufs=4) as pln,
            tc.tile_pool(name="pbc", bufs=2) as pbc,
            tc.tile_pool(name="pps", bufs=4, space="PSUM") as pps,
            tc.tile_pool(name="ppo", bufs=2, space="PSUM") as ppo,
            tc.tile_pool(name="pst", bufs=2, space="PSUM") as pst,
        ):
            xT = pc.tile([P, DS, T], F32)     # residual, transposed
            xb = pc.tile([P, DS, T], BF16)    # bf16 copy of residual (LN input)
            hT = pc.tile([P, DS, T], BF16)    # LN output; reused as attn y^T
            qkT = pc.tile([P, 2 * DS, T], BF16)  # q rows 0:8, k rows 8:16; reused as gT
            Vg = pc.tile([P, TS, H, HD + 1], BF16)
            yT = hT
            sm = pc.tile([P, SM_COLS], F32)
            mask = pc.tile([P, P], BF16)
            ones_row = pc.tile([1, NCH], BF16)
            ones_cb = pc.tile([P, 1], BF16)
            eps_t = pc.tile([1, 1], F32)

            nc.vector.memset(ones_row[:], 1.0)
            nc.vector.memset(ones_cb[:], 1.0)
            nc.vector.memset(eps_t[:], EPS)
            nc.vector.memset(Vg[:, :, :, HD:HD + 1], 1.0)
            nc.sync.dma_start(sm[:], sm_d[:, :])
            nc.sync.dma_start(mask[:], mask_d[:, :])

            def ln_stats_k(stat2c, c, k):
                """stat2c: PSUM [65, NCH]; row 0 accumulates sum, row 64 sumsq."""
                tch = bass.ts(c, NCH)
                nc.vector.tensor_copy(xb[:, k, tch], xT[:, k, tch])
                sq = px.tile([P, NCH], BF16, tag="sq")
                nc.vector.tensor_mul(sq[:], xb[:, k, tch], xb[:, k, tch])
                nc.tensor.matmul(stat2c[0:1, :], ones_cb[:],
                                 xb[:, k, tch], start=(k == 0),
                                 stop=(k == DS - 1), skip_group_check=True)
                nc.tensor.matmul(stat2c[64:65, :], ones_cb[:],
                                 sq[:], start=(k == 0), stop=(k == DS - 1),
                                 skip_group_check=True)

            def ln_finish(stat2c):
                """-> (A, B) bf16 [P, NCH] planes: h = x*A + B."""
                m2 = pr.tile([1, NCH], F32, tag="r")
                nc.scalar.activation(m2[:], stat2c[0:1, :],
                                     AF.Square, scale=1.0 / D)
                var = pr.tile([1, NCH], F32, tag="r")
                nc.vector.scalar_tensor_tensor(var[:], stat2c[64:65, :],
                                               1.0 / D, m2[:],
                                               op0=ALU.mult, op1=ALU.subtract)
                sd = pr.tile([1, NCH], F32, tag="r")
                nc.scalar.activation(sd[:], var[:], AF.Sqrt, bias=eps_t[:])
                srow = pr.tile([1, NCH], BF16, tag="rb")
                nm = pr.tile([1, NCH], BF16, tag="rb")
                with nc.allow_low_precision(reason="LN planes applied in bf16 anyway"):
                    nc.vector.reciprocal(srow[:], sd[:])
                    nc.vector.scalar_tensor_tensor(nm[:], stat2c[0:1, :],
                                                   -1.0 / D, srow[:],
                                                   op0=ALU.mult, op1=ALU.mult)
                A = pln.tile([P, NCH], BF16, tag="pl")
                nc.gpsimd.partition_broadcast(A[:], srow[:], channels=P)
                Bp = pln.tile([P, NCH], BF16, tag="pl")
                nc.gpsimd.partition_broadcast(Bp[:], nm[:], channels=P)
                return A, Bp

            def ln_apply(A, Bp, c):
                tch = bass.ts(c, NCH)
                for k in range(DS):
                    tmp = px.tile([P, NCH], BF16, tag="lt")
                    nc.vector.tensor_mul(tmp[:], xb[:, k, tch], A[:])
                    nc.vector.tensor_add(hT[:, k, tch], tmp[:], Bp[:])

            loop_cm = tc.For_i(0, repeat, 1) if repeat > 1 else contextlib.nullcontext()
            with loop_cm:
                nc.sync.dma_start(xT[:], x0T_d[:, :].rearrange("(po pi) t -> pi po t", pi=P))
                stat0 = pst.tile([65, NCH], F32, tag="st")
                stat1 = pst.tile([65, NCH], F32, tag="st")
                stat = [stat0, stat1]
                for c in range(2):
                    for k in range(DS):
                        ln_stats_k(stat[c], c, k)
                    A0c, B0c = ln_finish(stat[c])
                    ln_apply(A0c, B0c, c)

                for l in range(L):
                    base = l * SM_PER_LAYER
                    if biases:
                        br_t = pbr.tile([1, 2 * D], BF16, tag="br")
                        nc.sync.dma_start(br_t[:], brow_d[l:l + 1, :])

                    # ---- q^T / k^T projections (bias via K=1 matmul row) ----
                    for which, W_d in ((0, WqT_d), (1, WkT_d)):
                        qoff = which * DS
                        for half in range(2):
                            wsl = pw.tile([P, DS, NCH], BF16, tag="w")
                            nc.sync.dma_start(
                                wsl[:],
                                W_d[l].rearrange("(po pi) o -> pi po o", pi=P)[:, :, bass.ts(half, NCH)])
                            for c in range(2):
                                for m in range(4):
                                    mo = half * 4 + m
                                    ps0 = pps.tile([P, NCH], F32, tag="a")
                                    if biases:
                                        bsl = br_t[0:1, which * D + mo * P: which * D + (mo + 1) * P]
                                        nc.tensor.matmul(ps0[:], bsl, ones_row[:], start=True, stop=False)
                                    for k in range(DS):
                                        nc.tensor.matmul(ps0[:], wsl[:, k, bass.ts(m, P)],
                                                         hT[:, k, bass.ts(c, NCH)],
                                                         start=(not biases and k == 0), stop=(k == DS - 1))
                                    nc.scalar.activation(qkT[:, qoff + mo, bass.ts(c, NCH)], ps0[:], AF.Copy)

                    # ---- V projection: out[t, o] into Vg ----
                    for half in range(2):
                        wsl = pw.tile([P, DS, NCH], BF16, tag="w")
                        nc.sync.dma_start(
                            wsl[:],
                            WvT_d[l].rearrange("(po pi) o -> pi po o", pi=P)[:, :, bass.ts(half, NCH)])
                        if biases:
                            bvs = pbv.tile([1, NCH], BF16, tag="bvs")
                            nc.sync.dma_start(bvs[:], bv_d[:, l * D + half * NCH:l * D + (half + 1) * NCH])
                        for t_ in range(TS):
                            ps_t = pps.tile([P, NCH], F32, tag="a")
                            if biases:
                                nc.tensor.matmul(ps_t[:], ones_row[0:1, 0:P], bvs[:],
                                                 start=True, stop=False)
                            for k in range(DS):
                                nc.tensor.matmul(ps_t[:], hT[:, k, bass.ts(t_, P)],
                                                 wsl[:, k, :],
                                                 start=(not biases and k == 0), stop=(k == DS - 1))
                            nc.scalar.activation(
                                Vg[:, t_, 8 * half:8 * half + 8, 0:HD],
                                ps_t[:].rearrange("p (h d) -> p h d", d=HD), AF.Copy)

                    # ---- attention: scores(u) pipelined over PV/norm(u-1) ----
                    def attn_scores(h, c):
                        pbase = (h % 2) * HD
                        sub = h // 2
                        PT = ppt.tile([P, TS, NCH], BF16, tag="pt")
                        ntk = 4 * c + 4
                        for tk in range(ntk):
                            ls = max(0, tk * P - c * NCH)
                            w_ = NCH - ls
                            sT = pps.tile([P, NCH], F32, tag="a")
                            nc.tensor.matmul(
                                sT[:, :w_],
                                qkT[pbase:pbase + HD, DS + sub, bass.ts(tk, P)],
                                qkT[pbase:pbase + HD, sub, c * NCH + ls:(c + 1) * NCH],
                                start=True, stop=True)
                            nc.scalar.activation(PT[:, tk, ls:], sT[:, :w_], AF.Exp,
                                                 scale=float(SCALE))
                            if tk >= 4 * c:
                                nc.vector.tensor_mul(PT[:, tk, ls:ls + P],
                                                     PT[:, tk, ls:ls + P], mask[:])
                        return PT

                    def attn_pv(h, c, PT):
                        pbase = (h % 2) * HD
                        sub = h // 2
                        tch = bass.ts(c, NCH)
                        ntk = 4 * c + 4
                        po = ppo.tile([HD + 1, NCH], F32, tag="o")
                        for tk in range(ntk):
                            ls = max(0, tk * P - c * NCH)
                            nc.tensor.matmul(po[:, ls:], Vg[:, tk, h, :],
                                             PT[:, tk, ls:],
                                             start=(tk == 0), stop=(tk == ntk - 1))
                        dn = pr.tile([1, NCH], F32, tag="dn")
                        nc.vector.reciprocal(dn[:], po[HD:HD + 1, :])
                        bc = pbc.tile([HD, NCH], F32, tag="bc")
                        nc.gpsimd.partition_broadcast(bc[:], dn[:], channels=HD)
                        nc.vector.tensor_mul(yT[pbase:pbase + HD, sub, tch],
                                             po[0:HD, :], bc[:])

                    # o-proj weights + LN2 stat tiles (o-proj c0 groups are
                    # interleaved into the attention c1 units to keep PE fed
                    # while ACT works through the exp stream)
                    wo0 = pw.tile([P, DS, NCH], BF16, tag="w")
                    nc.sync.dma_start(wo0[:], WoT_d[l].rearrange("(po pi) o -> pi po o", pi=P)[:, :, bass.ts(0, NCH)])
                    wo1 = pw.tile([P, DS, NCH], BF16, tag="w")
                    nc.sync.dma_start(wo1[:], WoT_d[l].rearrange("(po pi) o -> pi po o", pi=P)[:, :, bass.ts(1, NCH)])
                    stat2a = pst.tile([65, NCH], F32, tag="st")
                    stat2b = pst.tile([65, NCH], F32, tag="st")
                    stat2 = [stat2a, stat2b]

                    def oproj_group(mo, c):
                        tch = bass.ts(c, NCH)
                        wsl_o = wo0 if mo < 4 else wo1
                        m = mo % 4
                        ps0 = pps.tile([P, NCH], F32, tag="a")
                        for k in range(DS):
                            nc.tensor.matmul(ps0[:], wsl_o[:, k, bass.ts(m, P)],
                                             yT[:, k, tch],
                                             start=(k == 0), stop=(k == DS - 1))
                        nc.vector.scalar_tensor_tensor(
                            xT[:, mo, tch], ps0[:],
                            sm[:, base + SM_BO + mo:base + SM_BO + mo + 1],
                            xT[:, mo, tch], op0=ALU.add, op1=ALU.add)
                        ln_stats_k(stat2[c], c, mo)

                    prev = None
                    for h in range(H):
                        PT = attn_scores(h, 0)
                        if prev is not None:
                            attn_pv(*prev)
                        prev = (h, 0, PT)
                    for h in range(H):
                        PT = attn_scores(h, 1)
                        attn_pv(*prev)
                        prev = (h, 1, PT)
                        if h % 2 == 1:
                            oproj_group(h // 2, 0)
                    attn_pv(*prev)
                    A2c, B2c = ln_finish(stat2[0])
                    ln_apply(A2c, B2c, 0)
                    for mo in range(DS):
                        oproj_group(mo, 1)
                    A2c, B2c = ln_finish(stat2[1])
                    ln_apply(A2c, B2c, 1)


                    # ---- MLP W1 -> gelu -> gT (qkT reused) ----
                    for quarter in range(4):
                        wsl = pw.tile([P, DS, NCH], BF16, tag="w")
                        nc.sync.dma_start(
                            wsl[:],
                            W1T_d[l].rearrange("(po pi) o -> pi po o", pi=P)[:, :, bass.ts(quarter, NCH)])
                        for c in range(2):
                            for m in range(4):
                                mo = quarter * 4 + m
                                ps0 = pps.tile([P, NCH], F32, tag="a")
                                for k in range(DS):
                                    nc.tensor.matmul(ps0[:], wsl[:, k, bass.ts(m, P)],
                                                     hT[:, k, bass.ts(c, NCH)],
                                                     start=(k == 0), stop=(k == DS - 1))
                                nc.scalar.activation(
                                    qkT[:, mo, bass.ts(c, NCH)], ps0[:], AF.Gelu,
                                    bias=sm[:, base + SM_B1 + mo:base + SM_B1 + mo + 1])

                    # ---- W2 + residual + next-LN stats ----
                    stat3a = pst.tile([65, NCH], F32, tag="st")
                    stat3b = pst.tile([65, NCH], F32, tag="st")
                    stat3 = [stat3a, stat3b]
                    for c in range(2):
                        tch = bass.ts(c, NCH)
                        for quarter in range(4):
                            w2q = pw.tile([P, D2S, P * 2], BF16, tag="w")
                            nc.sync.dma_start(
                                w2q[:],
                                W2T_d[l].rearrange("(po pi) o -> pi po o", pi=P)[:, :, bass.ts(quarter, P * 2)])
                            for m in range(2):
                                mo = quarter * 2 + m
                                ps0 = pps.tile([P, NCH], F32, tag="a")
                                for k in range(D2S):
                                    nc.tensor.matmul(ps0[:], w2q[:, k, bass.ts(m, P)],
                                                     qkT[:, k, tch],
                                                     start=(k == 0), stop=(k == D2S - 1))
                                nc.vector.scalar_tensor_tensor(
                                    xT[:, mo, tch], ps0[:],
                                    sm[:, base + SM_B2 + mo:base + SM_B2 + mo + 1],
                                    xT[:, mo, tch], op0=ALU.add, op1=ALU.add)
                                ln_stats_k(stat3[c], c, mo)
                        A3c, B3c = ln_finish(stat3[c])
                        ln_apply(A3c, B3c, c)

                # ---- vocab head (final LN already applied in last W2 pass) ----
                hw_r = hT_w_d[:, :].rearrange("(po pi) v -> pi po v", pi=P)
                for vp in range(VS // 2):
                    ws0 = pw.tile([P, DS, NCH], BF16, tag="w")
                    nc.sync.dma_start(ws0[:], hw_r[:, :, bass.ts(2 * vp, NCH)])
                    ws1 = pw.tile([P, DS, NCH], BF16, tag="w")
                    nc.sync.dma_start(ws1[:], hw_r[:, :, bass.ts(2 * vp + 1, NCH)])
                    if biases:
                        hb2 = phb.tile([1, 2 * NCH], BF16, tag="hb")
                        nc.sync.dma_start(hb2[:], hb_d[:, 2 * vp * NCH:(2 * vp + 2) * NCH])
                    for t_ in range(TS):
                        ps0 = pps.tile([P, NCH], F32, tag="a")
                        ps1 = pps.tile([P, NCH], F32, tag="a")
                        if biases:
                            nc.tensor.matmul(ps0[:], ones_row[0:1, 0:P], hb2[0:1, 0:NCH],
                                             start=True, stop=False)
                            nc.tensor.matmul(ps1[:], ones_row[0:1, 0:P], hb2[0:1, NCH:2 * NCH],
                                             start=True, stop=False)
                        for k in range(DS):
                            nc.tensor.matmul(ps0[:], hT[:, k, bass.ts(t_, P)],
                                             ws0[:, k, :],
                                             start=(not biases and k == 0), stop=(k == DS - 1))
                            nc.tensor.matmul(ps1[:], hT[:, k, bass.ts(t_, P)],
                                             ws1[:, k, :],
                                             start=(not biases and k == 0), stop=(k == DS - 1))
                        for j, psx in ((0, ps0), (1, ps1)):
                            ot = px.tile([P, NCH], F32, tag="ot")
                            nc.scalar.activation(ot[:], psx[:], AF.Copy)
                            nc.sync.dma_start(out_r[:, t_, bass.ts(2 * vp + j, NCH)], ot[:])

    nc.compile()
    return nc


_NC = {}
_BIASES = True


def _get_nc(repeat=1):
    key = (repeat, _BIASES)
    if key not in _NC:
        _NC[key] = _build(repeat, biases=_BIASES)
    return _NC[key]


def _pack_cols(vec, ncols):
    """[ncols*128] -> [128, ncols] with column j = vec[j*128:(j+1)*128]."""
    return np.ascontiguousarray(vec.reshape(ncols, P).T)


def kernel(idx, timesteps, tok_emb_w, pos_emb, global_pos_emb,
           ln1_w, ln1_b, Wq, bq, Wk, bk, Wv, bv, Wo, bo,
           ln2_w, ln2_b, W1, b1, W2, b2, lnf_w, lnf_b, head_w):
    global LAST_RESULTS
    f = lambda a: np.asarray(a, dtype=np.float32)
    idx = np.asarray(idx, dtype=np.int64)
    tsteps = np.asarray(timesteps, dtype=np.int64)
    tok_emb_w, pos_emb, global_pos_emb = f(tok_emb_w), f(pos_emb), f(global_pos_emb)

    # embedding on host (tiny compute, avoids on-device gather)
    x0 = tok_emb_w[idx] + global_pos_emb[0][tsteps[:, 0]][:, None, :] + pos_emb[:, :T]
    x0 = np.ascontiguousarray(x0.astype(np.float32))

    # fold LN affine params into adjacent weights (exact rewrite)
    Wq, bq, Wk, bk, Wv, bv = f(Wq), f(bq), f(Wk), f(bk), f(Wv), f(bv)
    Wo, bo, W1, b1, W2, b2 = f(Wo), f(bo), f(W1), f(b1), f(W2), f(b2)
    ln1_w, ln1_b, ln2_w, ln2_b = f(ln1_w), f(ln1_b), f(ln2_w), f(ln2_b)
    lnf_w, lnf_b, head_w = f(lnf_w), f(lnf_b), f(head_w)

    Wqf = Wq * ln1_w[:, None, :]
    bqf = bq + np.einsum('lod,ld->lo', Wq, ln1_b)
    Wkf = Wk * ln1_w[:, None, :]
    bkf = bk + np.einsum('lod,ld->lo', Wk, ln1_b)
    Wvf = Wv * ln1_w[:, None, :]
    bvf = bv + np.einsum('lod,ld->lo', Wv, ln1_b)
    W1f = W1 * ln2_w[:, None, :]
    b1f = b1 + np.einsum('lod,ld->lo', W1, ln2_b)
    headf = head_w * lnf_w[None, :]
    hb = head_w @ lnf_b

    global _BIASES
    _BIASES = bool(np.any(bqf) or np.any(bkf) or np.any(bvf) or np.any(hb))

    bf = lambda a: np.ascontiguousarray(np.asarray(a, np.float32)).astype(ml_dtypes.bfloat16)
    shared = {
        "WqT": bf(Wqf.transpose(0, 2, 1)),
        "WkT": bf(Wkf.transpose(0, 2, 1)),
        "WvT": bf(Wvf.transpose(0, 2, 1)),
        "WoT": bf(Wo.transpose(0, 2, 1)),
        "W1T": bf(W1f.transpose(0, 2, 1)),
        "W2T": bf(W2.transpose(0, 2, 1)),
        "headT": bf(headf.T),
        "bvB": bf(bvf.reshape(1, L * D)),
        "hbB": bf(hb.reshape(1, V)),
        "brows": bf(np.concatenate([bqf, bkf], axis=1)),  # [L, 2D]
    }
    smalls = np.zeros((P, SM_COLS), np.float32)
    for l in range(L):
        b_ = l * SM_PER_LAYER
        smalls[:, b_ + SM_BO:b_ + SM_BO + 8] = _pack_cols(bo[l], DS)
        smalls[:, b_ + SM_B2:b_ + SM_B2 + 8] = _pack_cols(b2[l], DS)
        smalls[:, b_ + SM_B1:b_ + SM_B1 + 16] = _pack_cols(b1f[l], D2S)
    shared["smalls"] = smalls

    m01 = (np.arange(P)[:, None] <= np.arange(P)[None, :])
    shared["mask01"] = m01.astype(ml_dtypes.bfloat16)

    in_maps = []
    for b_ in range(B):
        m = dict(shared)
        m["x0T"] = np.ascontiguousarray(x0[b_].T)
        in_maps.append(m)

    global LAST_IN_MAPS
    LAST_IN_MAPS = in_maps
    nc = _get_nc()
    res = run_bass_kernel_spmd(nc, in_maps, core_ids=list(range(B)), trace=TRACE)
    LAST_RESULTS = res
    out = np.stack([np.asarray(res.results[c]["logits"], np.float32) for c in range(B)])
    return out


# ---------------------------------------------------------------------------
# Timing helpers (test-only): replicate run_bass_via_pjrt's sharded jit with
# device-resident inputs so repeated calls measure (dispatch + NEFF exec).
# ---------------------------------------------------------------------------
LAST_IN_MAPS = None


def _sharded_exec(nc, in_maps):
    import jax
    from jax.experimental.shard_map import shard_map
    from jax.sharding import Mesh, PartitionSpec
    from concourse import bass2jax

    bass2jax.install_neuronx_cc_hook()
    n_cores = len(in_maps)
    partition_name = nc.partition_id_tensor.name if nc.partition_id_tensor else None
    in_names, out_names, out_avals, zero_outs = [], [], [], []
    for alloc in nc.m.functions[0].allocations:
        if not isinstance(alloc, mybir.MemoryLocationSet):
            continue
        name = alloc.memorylocations[0].name
        if alloc.kind == "ExternalInput":
            if name != partition_name:
                in_names.append(name)
        elif alloc.kind == "ExternalOutput":
            shape = tuple(alloc.tensor_shape)
            dtype = mybir.dt.np(alloc.dtype)
            out_names.append(name)
            out_avals.append(jax.core.ShapedArray(shape, dtype))
            zero_outs.append(np.zeros(shape, dtype))
    n_params = len(in_names)
    n_outs = len(out_avals)
    all_in_names = list(in_names) + list(out_names)
    if partition_name is not None:
        all_in_names.append(partition_name)
    donate = tuple(range(n_params, n_params + n_outs))

    def _body(*args):
        operands = list(args)
        if partition_name is not None:
            operands.append(bass2jax.partition_id_tensor())
        outs = bass2jax._bass_exec_p.bind(
            *operands,
            out_avals=tuple(out_avals),
            in_names=tuple(all_in_names),
            out_names=tuple(out_names),
            lowering_input_output_aliases=(),
            sim_require_finite=True,
            sim_require_nnan=True,
            nc=nc,
        )
        return tuple(outs)

    devices = jax.devices()[:n_cores]
    mesh = Mesh(np.asarray(devices), ("core",))
    sharded = jax.jit(
        shard_map(_body, mesh=mesh,
                  in_specs=(PartitionSpec("core"),) * (n_params + n_outs),
                  out_specs=(PartitionSpec("core"),) * n_outs,
                  check_rep=False),
        donate_argnums=donate, keep_unused=True)

    concat_in = [np.concatenate([np.asarray(m[name]) for m in in_maps], axis=0)
                 for name in in_names]
    concat_zeros = [np.zeros((n_cores * z.shape[0], *z.shape[1:]), z.dtype)
                    for z in zero_outs]
    from jax.sharding import NamedSharding
    sh = NamedSharding(mesh, PartitionSpec("core"))
    dev_in = [jax.device_put(a, sh) for a in concat_in]
    return sharded, dev_in, concat_zeros, sh


def _time_exec(nc, in_maps, iters):
    import time as _time
    import jax
    sharded, dev_in, concat_zeros, sh = _sharded_exec(nc, in_maps)
    times = []
    for _ in range(iters):
        zs = [jax.device_put(z, sh) for z in concat_zeros]
        jax.block_until_ready(zs)
        jax.block_until_ready(dev_in)
        t0 = _time.perf_counter()
        out = sharded(*dev_in, *zs)
        jax.block_until_ready(out)
        times.append(_time.perf_counter() - t0)
    return times


def timed_run(iters=5):
    assert LAST_IN_MAPS is not None, "call kernel() first"
    return _time_exec(_get_nc(), LAST_IN_MAPS, iters)


def timed_slope(ns=(1, 4, 12), zsets=12):
    """Async-dispatch n calls back-to-back; slope of total-time vs n ~ exec."""
    import time as _time
    import jax
    assert LAST_IN_MAPS is not None
    sharded, dev_in, concat_zeros, sh = _sharded_exec(_get_nc(), LAST_IN_MAPS)
    all_zs = [[jax.device_put(z, sh) for z in concat_zeros] for _ in range(zsets)]
    jax.block_until_ready(all_zs)
    jax.block_until_ready(dev_in)
    # warm
    out = sharded(*dev_in, *all_zs[0])
    jax.block_until_ready(out)
    res = {}
    for n in ns:
        zs_fresh = [[jax.device_put(z, sh) for z in concat_zeros] for _ in range(n)]
        jax.block_until_ready(zs_fresh)
        t0 = _time.perf_counter()
        outs = [sharded(*dev_in, *zs_fresh[i]) for i in range(n)]
        jax.block_until_ready(outs)
        res[n] = _time.perf_counter() - t0
    return res


def timed_repeat(r=5, iters=6):
    """exec_ns ~= (min_time(R=r NEFF) - min_time(R=1 NEFF)) / (r-1)."""
    assert LAST_IN_MAPS is not None
    t1 = min(_time_exec(_get_nc(1), LAST_IN_MAPS, iters))
    tr = min(_time_exec(_get_nc(r), LAST_IN_MAPS, iters))
    return (tr - t1) / (r - 1), t1, tr
